# revision 22
# baseline (speedup 1.0000x reference)
"""DiffNet GNN message-passing kernel for 8 Trainium2 NeuronCores.

Math: final_user = t2/deg_soc + 2*h1 + t3/deg_info, restricted to batch users,
where h1 = A_soc@u0/deg_soc + u0 (needed for ALL users since layer 2 gathers
arbitrary columns), t2 = A_soc@h1 at batch rows only, t3 = A_info@item_emb at
batch rows only. Output = sigmoid(2 * sum(final_user[uids] * item_emb[iids])).

Sharding: by output row range (12500 users/core). Layer-1 SpMM over the full
edge set partitions exactly by row; chunked AllGathers publish h1 (bf16,
128-wide padded rows so each gather element is 256B); layer-2 and info SpMMs
run only on each core's batch-user rows.

Device SpMM: edges sorted by (group, col-chunk, tile); dma_gather pulls 256B
message rows (padded bf16) from HBM tables directly into bf16 SBUF tiles;
SWDGE descriptor generation is spread across all 4 gpsimd queue core-pairs
(queue_num round-robin); a one-hot matrix (batched DVE is_equal) + PE matmul
(messages stationary [128e,64d], one-hot moving [128e,128r]) does the
segment-sum into PSUM; PE transpose + fused scale/residual produce rows.
"""

import sys

sys.path.insert(0, "/opt/trn_rl_repo")

import math
import os

import numpy as np
import ml_dtypes

import concourse.bacc as bacc
import concourse.bass as bass
import concourse.mybir as mybir
import concourse.tile as tile
from concourse.masks import make_identity
from concourse.bass_utils import run_bass_kernel_spmd

P = 128
BF16 = ml_dtypes.bfloat16
GSUB = 8192  # max idxs per dma_gather sub-call
EPAD = 128  # padded table row width (bf16) -> 256B row stride


def _dma_gather128(gp, out_ap, in_ap, idxs_ap, num_idxs, queue_num):
    """dma_gather of 128B (64 x bf16) elements from a 256B-stride table.

    Mirrors bass.BassGpSimd.dma_gather's HBM-source path but skips its
    elem_size_bytes % 256 == 0 assert: that restriction belongs to the
    transpose (xbar) path; the plain path emits per-index descriptors of
    arbitrary length. The table row stride must still be a 256B multiple
    (stride_bytes_256 is an 8-bit descriptor field), which the EPAD-wide
    tables satisfy.
    """
    d = 64
    assert in_ap.ap[0][0] == EPAD  # stride 128 elems = 256B
    assert in_ap.ap[-1][1] == d and out_ap.ap[-1][1] == d
    inst = gp.add_instruction(
        mybir.InstDMAGatherAnt(
            name=gp.bass.get_next_instruction_name(),
            ins=[
                *gp.lower_ap_dma(in_ap, for_custom_bir_dma=True),
                gp.lower_ap(idxs_ap),
                gp.lower_val_access(gp.to_reg(num_idxs)),
            ],
            outs=[gp.lower_ap(out_ap)],
            transpose=False,
            num_idxs=num_idxs,
            elem_size=d,
            stride_bytes_256=1,
            gen_mode=0,
            single_packet=False,
            queue_num=queue_num,
            sbuf_tokens_per_rank=0,
            sbuf_free_dim_per_rank=0,
            sbuf_free_dim_pad_per_rank=0,
            sbuf_byte_offset=0,
        )
    )
    return inst


class Cfg:
    def __init__(self, n_user, n_item, d, n_cores, tpg1, gpa, tpg2, chunk):
        self.n_user = n_user
        self.n_item = n_item
        self.d = d
        self.nc = n_cores
        self.rpc = n_user // n_cores  # rows per core
        t1 = -(-self.rpc // P)  # L1 tiles per core (unpadded)
        self.tpg1 = tpg1  # L1 tiles per group
        self.t1p = -(-t1 // tpg1) * tpg1  # padded L1 tile count
        self.ng1 = self.t1p // tpg1
        self.gpa = gpa  # groups per AllGather chunk
        assert self.ng1 % gpa == 0
        self.agc = self.ng1 // gpa  # number of AG chunks
        self.cr = tpg1 * P * gpa  # rows per core per AG chunk
        self.shard_rows = self.t1p * P
        self.flat_h1 = self.nc * self.shard_rows  # flat h1 rows
        self.tpg2 = tpg2  # batch tiles per group (L2 & info)
        self.chunk = chunk  # max gather-chunk rows (int16 limit)
        self.nch_u = -(-n_user // chunk)
        self.ch_u = -(-n_user // self.nch_u)
        self.nch_i = -(-n_item // chunk)
        self.ch_i = -(-n_item // self.nch_i)
        # L2 gathers directly from the per-AG-chunk h1 tables
        self.nch_h = self.agc
        self.ch_h = self.nc * self.cr
        assert self.ch_h < 2 ** 15


REAL = Cfg(100000, 50000, 64, 8, 7, 2, 4, 25088)


def _wrap_idx(idx_call):
    """[n] int16 -> [128, n/16] wrapped+replicated."""
    n = idx_call.shape[0]
    a = idx_call.reshape(n // 16, 16).T  # [16, n/16]
    return np.tile(a, (8, 1))


class SpmmSched:
    """SPMD-uniform slot/block layout for one SpMM (same across cores)."""

    def __init__(self, ntp, tpg, nch):
        self.ntp = ntp  # padded tile count
        self.tpg = tpg
        self.ng = ntp // tpg
        self.nch = nch
        self.cap = None  # [ntp, nch] slots, multiples of 128

    def finalize(self):
        ntp, tpg, ng, nch = self.ntp, self.tpg, self.ng, self.nch
        cap = self.cap
        # ensure every tile has >=1 block so its PSUM region gets zeroed
        for t in range(ntp):
            if cap[t].sum() == 0:
                cap[t, 0] = P
        # region = (g, c): tiles g*tpg..g*tpg+tpg-1
        self.sub_off = np.zeros((ntp, nch), np.int64)  # slot offset in region
        self.region_nidx = np.zeros((ng, nch), np.int64)
        self.slot_base = np.zeros((ng, nch), np.int64)  # global slot offset
        self.blk_base = np.zeros((ng, nch), np.int64)
        self.group_blk0 = np.zeros(ng, np.int64)
        s = 0
        b = 0
        for g in range(ng):
            self.group_blk0[g] = b
            for c in range(nch):
                self.slot_base[g, c] = s
                self.blk_base[g, c] = b
                off = 0
                for tl in range(tpg):
                    t = g * tpg + tl
                    self.sub_off[t, c] = off
                    off += cap[t, c]
                self.region_nidx[g, c] = off
                s += off
                b += off // P
        self.total_slots = s
        self.total_blocks = b
        self.group_blocks = [
            int(sum(self.region_nidx[g]) // P) for g in range(ng)
        ]
        # per (g, tl): ordered list of global block ids (for start/stop flags)
        self.tile_blocks = {}
        for g in range(ng):
            for tl in range(self.tpg):
                t = g * self.tpg + tl
                blks = []
                for c in range(nch):
                    b0 = self.blk_base[g, c] + self.sub_off[t, c] // P
                    blks += list(range(b0, b0 + cap[t, c] // P))
                self.tile_blocks[(g, tl)] = blks
        # idx array column offsets (global, in units of 16 slots)
        self.idx_off = np.zeros((ng, nch), np.int64)
        w = 0
        for g in range(ng):
            for c in range(nch):
                self.idx_off[g, c] = w
                w += self.region_nidx[g, c] // 16
        self.idx_w = w


def _sched_caps(sched, per_core_tc_counts):
    """per_core_tc_counts: list of [ntp, nch] arrays -> set caps."""
    mx = np.maximum.reduce(per_core_tc_counts)
    sched.cap = (-(-mx // P) * P).astype(np.int64)
    sched.finalize()


def _fill_spmm(sched, rows_t, cols_c, col_idx, rowloc, vals):
    """Place one core's edges into the schedule's slot space.

    rows_t: tile id per edge; cols_c: chunk id; col_idx: int16 local col;
    rowloc: row-in-tile (0..127); vals: edge values (float32).
    Returns (idx_arr [128, idx_w] i16, rl [128, B] bf16, val_w [128, B] f32).
    """
    ntp, tpg, ng, nch = sched.ntp, sched.tpg, sched.ng, sched.nch
    g_e = rows_t // tpg
    tl_e = rows_t % tpg
    bid = (g_e * nch + cols_c) * tpg + tl_e
    order = np.argsort(bid, kind="stable")
    bid_s = bid[order]
    counts = np.bincount(bid_s, minlength=ng * nch * tpg)
    starts = np.concatenate([[0], np.cumsum(counts)[:-1]])
    rank = np.arange(len(bid_s)) - starts[bid_s]
    t_s = rows_t[order]
    c_s = cols_c[order]
    g_s = g_e[order]
    slot = (
        sched.slot_base[g_s, c_s]
        + sched.sub_off[t_s, c_s]
        + rank
    )
    ns = sched.total_slots
    idx_flat = np.zeros(ns, np.int32)
    rl_flat = np.full(ns, -1.0, np.float32)
    val_flat = np.zeros(ns, np.float32)
    idx_flat[slot] = col_idx[order]
    rl_flat[slot] = rowloc[order]
    val_flat[slot] = vals[order]
    # pad slots keep idx 0 (real harmless gathers) so every slot is always
    # written -- avoids NaN garbage flowing into the matmul.
    idx_arr = np.empty((P, sched.idx_w), np.int16)
    for g in range(ng):
        for c in range(nch):
            n = sched.region_nidx[g, c]
            if n == 0:
                continue
            s0 = sched.slot_base[g, c]
            w0 = sched.idx_off[g, c]
            idx_arr[:, w0 : w0 + n // 16] = _wrap_idx(
                idx_flat[s0 : s0 + n].astype(np.int16)
            )
    rl = np.ascontiguousarray(
        rl_flat.reshape(sched.total_blocks, P).T.astype(BF16)
    )
    val_w = np.ascontiguousarray(val_flat.reshape(sched.total_blocks, P).T)
    return idx_arr, rl, val_w


def _prep(cfg, inputs):
    """All host-side preprocessing. Returns (plan, in_maps, out_meta)."""
    nc_, d = cfg.nc, cfg.d
    user_emb = np.asarray(inputs["user_emb"], np.float32)
    item_emb = np.asarray(inputs["item_emb"], np.float32)
    s_rows = np.asarray(inputs["social_rows"], np.int64)
    s_cols = np.asarray(inputs["social_cols"], np.int64)
    s_vals = np.asarray(inputs["social_vals"], np.float32)
    i_rows = np.asarray(inputs["info_rows"], np.int64)
    i_cols = np.asarray(inputs["info_cols"], np.int64)
    i_vals = np.asarray(inputs["info_vals"], np.float32)
    uids = np.asarray(inputs["user_ids"], np.int64)
    iids = np.asarray(inputs["item_ids"], np.int64)
    eps = 1e-8

    ones = bool(np.all(s_vals == 1.0) and np.all(i_vals == 1.0))

    deg_soc = np.bincount(s_rows, weights=s_vals, minlength=cfg.n_user)
    deg_info = np.bincount(i_rows, weights=i_vals, minlength=cfg.n_user)
    inv_soc = (1.0 / (deg_soc.astype(np.float32) + eps)).astype(np.float32)
    inv_info = (1.0 / (deg_info.astype(np.float32) + eps)).astype(np.float32)

    # padded bf16 gather tables (256B rows)
    user_bf = np.zeros((cfg.n_user, EPAD), BF16)
    user_bf[:, :d] = user_emb.astype(BF16)
    item_bf = np.zeros((cfg.n_item, EPAD), BF16)
    item_bf[:, :d] = item_emb.astype(BF16)

    # batch users
    uniq = np.unique(uids)
    owner = uniq // cfg.rpc
    bu = [uniq[owner == c] for c in range(nc_)]
    ubmax = max(len(b) for b in bu)
    ubt = -(-ubmax // P)
    ng2 = max(1, -(-ubt // cfg.tpg2))
    ubt_p = ng2 * cfg.tpg2
    ubp = ubt_p * P

    # --- L1 schedule ---
    s1 = SpmmSched(cfg.t1p, cfg.tpg1, cfg.nch_u)
    order = np.argsort(s_rows, kind="stable")
    sr, sc, sv = s_rows[order], s_cols[order], s_vals[order]
    bounds = np.searchsorted(sr, [c * cfg.rpc for c in range(nc_ + 1)])
    core_l1 = []
    tc_counts = []
    for c in range(nc_):
        lo, hi = bounds[c], bounds[c + 1]
        lr = sr[lo:hi] - c * cfg.rpc
        col = sc[lo:hi]
        t = lr // P
        ch = col // cfg.ch_u
        core_l1.append((t, ch, (col - ch * cfg.ch_u), lr % P, sv[lo:hi]))
        m = np.zeros((cfg.t1p, cfg.nch_u), np.int64)
        np.add.at(m, (t, ch), 1)
        tc_counts.append(m)
    _sched_caps(s1, tc_counts)

    # --- L2 & info: batch-row-restricted ---
    slot_of = np.full(cfg.n_user, -1, np.int64)
    for c in range(nc_):
        slot_of[bu[c]] = np.arange(len(bu[c]))
    in_batch = slot_of >= 0

    def batch_edges(rows, cols, vals):
        m = in_batch[rows]
        r, co, v = rows[m], cols[m], vals[m]
        core = r // cfg.rpc
        return r, co, v, core

    s2 = SpmmSched(ubt_p, cfg.tpg2, cfg.nch_h)
    si = SpmmSched(ubt_p, cfg.tpg2, cfg.nch_i)

    def h1_flat(col):
        own = col // cfg.rpc
        lr = col - own * cfg.rpc
        k = lr // cfg.cr
        off = lr - k * cfg.cr
        return k * (nc_ * cfg.cr) + own * cfg.cr + off

    r2, c2, v2, core2 = batch_edges(sr, sc, sv)
    f2 = h1_flat(c2)
    ri, ci, vi, corei = batch_edges(i_rows, i_cols, i_vals)

    core_l2, core_in = [], []
    tc2, tci = [], []
    for c in range(nc_):
        m = core2 == c
        sl = slot_of[r2[m]]
        t = sl // P
        ch = f2[m] // cfg.ch_h
        core_l2.append((t, ch, f2[m] - ch * cfg.ch_h, sl % P, v2[m]))
        a = np.zeros((ubt_p, cfg.nch_h), np.int64)
        np.add.at(a, (t, ch), 1)
        tc2.append(a)
        m = corei == c
        sl = slot_of[ri[m]]
        t = sl // P
        ch = ci[m] // cfg.ch_i
        core_in.append((t, ch, ci[m] - ch * cfg.ch_i, sl % P, vi[m]))
        a = np.zeros((ubt_p, cfg.nch_i), np.int64)
        np.add.at(a, (t, ch), 1)
        tci.append(a)
    _sched_caps(s2, tc2)
    _sched_caps(si, tci)

    # --- final pairs ---
    pcore = uids // cfg.rpc
    pch = iids // cfg.ch_i
    fcap = np.zeros(cfg.nch_i, np.int64)
    per_core_pairs = []
    for c in range(nc_):
        m = np.nonzero(pcore == c)[0]
        o = m[np.argsort(pch[m], kind="stable")]
        per_core_pairs.append(o)
        cnts = np.bincount(pch[o], minlength=cfg.nch_i)
        fcap = np.maximum(fcap, cnts)
    fcap = -(-fcap // P) * P
    fcap = np.maximum(fcap, P)
    fbase = np.concatenate([[0], np.cumsum(fcap)])
    ftot = int(fbase[-1])

    plan = dict(
        cfg=cfg, s1=s1, s2=s2, si=si, ubt_p=ubt_p, ubp=ubp, ng2=ng2,
        fcap=fcap, fbase=fbase, ftot=ftot, ones=ones,
    )

    in_maps = []
    out_meta = []  # per core: (pair_js, slots)
    for c in range(nc_):
        t, ch, cidx, rl, v = core_l1[c]
        l1_idx, l1_rl, l1_val = _fill_spmm(s1, t, ch, cidx, rl, v)
        t, ch, cidx, rl, v = core_l2[c]
        l2_idx, l2_rl, l2_val = _fill_spmm(s2, t, ch, cidx, rl, v)
        t, ch, cidx, rl, v = core_in[c]
        in_idx, in_rl, in_val = _fill_spmm(si, t, ch, cidx, rl, v)

        # u0 shard
        u0s = np.zeros((cfg.shard_rows, d), np.float32)
        nrow = min(cfg.rpc, cfg.n_user - c * cfg.rpc)
        u0s[:nrow] = user_emb[c * cfg.rpc : c * cfg.rpc + nrow]

        # invdeg arrays
        ist = np.zeros((P, cfg.t1p), np.float32)
        rows = c * cfg.rpc + np.arange(nrow)
        ist[np.arange(nrow) % P, np.arange(nrow) // P] = inv_soc[rows]
        isb = np.zeros((P, ubt_p), np.float32)
        iib = np.zeros((P, ubt_p), np.float32)
        nb = len(bu[c])
        isb[np.arange(nb) % P, np.arange(nb) // P] = inv_soc[bu[c]]
        iib[np.arange(nb) % P, np.arange(nb) // P] = inv_info[bu[c]]

        # h1 batch gather idx (local shard rows); pads gather row 0
        h1b = np.zeros(ubp, np.int16)
        h1b[:nb] = (bu[c] - c * cfg.rpc).astype(np.int16)

        # final pairs
        o = per_core_pairs[c]
        pu = np.zeros(ftot, np.int16)
        pi = np.zeros(ftot, np.int16)
        slots = np.empty(len(o), np.int64)
        pos = 0
        for chn in range(cfg.nch_i):
            sel = o[pch[o] == chn]
            k = len(sel)
            s0 = fbase[chn]
            pu[s0 : s0 + k] = slot_of[uids[sel]].astype(np.int16)
            pi[s0 : s0 + k] = (iids[sel] - chn * cfg.ch_i).astype(np.int16)
            slots[pos : pos + k] = s0 + np.arange(k)
            pos += k
        out_meta.append((o, slots))

        m = {
            "user_bf": user_bf,
            "item_bf": item_bf,
            "u0s": u0s,
            "l1_idx": l1_idx, "l1_rl": l1_rl,
            "l2_idx": l2_idx, "l2_rl": l2_rl,
            "in_idx": in_idx, "in_rl": in_rl,
            "ist": ist, "isb": isb, "iib": iib,
            "h1b_idx": _wrap_idx(h1b),
            "pu_idx": _wrap_idx(pu), "pi_idx": _wrap_idx(pi),
        }
        if not ones:
            m["l1_val"] = l1_val
            m["l2_val"] = l2_val
            m["in_val"] = in_val
        in_maps.append(m)
    return plan, in_maps, out_meta


def _build_program(plan):
    cfg = plan["cfg"]
    s1, s2, si = plan["s1"], plan["s2"], plan["si"]
    ubt_p, ubp, ng2 = plan["ubt_p"], plan["ubp"], plan["ng2"]
    fcap, fbase, ftot = plan["fcap"], plan["fbase"], plan["ftot"]
    ones = plan["ones"]
    d = cfg.d
    nc_ = cfg.nc
    f32 = mybir.dt.float32
    bf = mybir.dt.bfloat16

    # SWDGE descriptor carveout: max descs per DMA instruction is
    # dynamic_dma_scratch_size/16; gathers are split into GSUB-idx sub-calls.
    nc = bacc.Bacc("TRN2", debug=False, num_devices=nc_, num_swdge_queues=4,
                   dynamic_dma_scratch_size=32768)
    qrr = {"q": 0}

    def next_q():
        q = qrr["q"]
        qrr["q"] = (q + 1) % 4
        return q

    t_userbf = nc.dram_tensor("user_bf", [cfg.n_user, EPAD], bf, kind="ExternalInput")
    t_itembf = nc.dram_tensor("item_bf", [cfg.n_item, EPAD], bf, kind="ExternalInput")
    t_u0s = nc.dram_tensor("u0s", [cfg.shard_rows, d], f32, kind="ExternalInput")
    t_l1i = nc.dram_tensor("l1_idx", [P, s1.idx_w], mybir.dt.int16, kind="ExternalInput")
    t_l1r = nc.dram_tensor("l1_rl", [P, s1.total_blocks], bf, kind="ExternalInput")
    t_l2i = nc.dram_tensor("l2_idx", [P, s2.idx_w], mybir.dt.int16, kind="ExternalInput")
    t_l2r = nc.dram_tensor("l2_rl", [P, s2.total_blocks], bf, kind="ExternalInput")
    t_ini = nc.dram_tensor("in_idx", [P, si.idx_w], mybir.dt.int16, kind="ExternalInput")
    t_inr = nc.dram_tensor("in_rl", [P, si.total_blocks], bf, kind="ExternalInput")
    t_ist = nc.dram_tensor("ist", [P, cfg.t1p], f32, kind="ExternalInput")
    t_isb = nc.dram_tensor("isb", [P, ubt_p], f32, kind="ExternalInput")
    t_iib = nc.dram_tensor("iib", [P, ubt_p], f32, kind="ExternalInput")
    t_h1bi = nc.dram_tensor("h1b_idx", [P, ubp // 16], mybir.dt.int16, kind="ExternalInput")
    t_pui = nc.dram_tensor("pu_idx", [P, ftot // 16], mybir.dt.int16, kind="ExternalInput")
    t_pii = nc.dram_tensor("pi_idx", [P, ftot // 16], mybir.dt.int16, kind="ExternalInput")
    t_scores = nc.dram_tensor("scores", [P, ftot // P], f32, kind="ExternalOutput")
    t_vals = {}
    if not ones:
        t_vals["l1"] = nc.dram_tensor("l1_val", [P, s1.total_blocks], f32, kind="ExternalInput")
        t_vals["l2"] = nc.dram_tensor("l2_val", [P, s2.total_blocks], f32, kind="ExternalInput")
        t_vals["in"] = nc.dram_tensor("in_val", [P, si.total_blocks], f32, kind="ExternalInput")

    with tile.TileContext(nc) as tc:
        with (
            tc.tile_pool(name="const", bufs=1) as cp,
            tc.tile_pool(name="persist", bufs=1) as pp,
            tc.tile_pool(name="idx", bufs=5) as idxp,
            tc.tile_pool(name="msgsbf", bufs=6) as mbp,
            tc.tile_pool(name="oh", bufs=3) as ohp,
            tc.tile_pool(name="rl", bufs=4) as rlp,
            tc.tile_pool(name="u0t", bufs=4) as u0p,
            tc.tile_pool(name="drain", bufs=2) as drp,
            tc.tile_pool(name="hrow", bufs=4) as hp,
            tc.tile_pool(name="psacc", bufs=5, space="PSUM") as pap,
            tc.tile_pool(name="pstr", bufs=2, space="PSUM") as ptp,
            tc.tile_pool(name="dram", bufs=1, space="DRAM") as dram,
        ):
            # ---- constants / persistent ----
            iota_i = cp.tile([P, P], mybir.dt.int32, tag="iotai")
            nc.gpsimd.iota(iota_i[:], pattern=[[1, P]], base=0, channel_multiplier=0)
            iota_bf = cp.tile([P, P], bf, tag="iotabf")
            nc.vector.tensor_copy(iota_bf[:], iota_i[:])
            ident = cp.tile([d, d], f32, tag="ident")
            make_identity(nc, ident[:])
            ist_t = pp.tile([P, cfg.t1p], f32, tag="ist")
            nc.sync.dma_start(ist_t[:], t_ist.ap())
            isb_t = pp.tile([P, ubt_p], f32, tag="isb")
            nc.sync.dma_start(isb_t[:], t_isb.ap())
            iib_t = pp.tile([P, ubt_p], f32, tag="iib")
            nc.sync.dma_start(iib_t[:], t_iib.ap())
            t3T = pp.tile([d, ubp], f32, tag="t3T")
            h1b_t = pp.tile([P, ubt_p, d], f32, tag="h1b")
            nc.vector.memzero(h1b_t[:])

            # internal DRAM
            h1ag = [
                dram.tile([cfg.cr, EPAD], bf, tag=f"h1ag{k}", name=f"h1ag{k}")
                for k in range(cfg.agc)
            ]
            h1fb = [
                dram.tile([nc_ * cfg.cr, EPAD], bf, tag=f"h1fb{k}",
                          name=f"h1fb{k}")
                for k in range(cfg.agc)
            ]
            h1_shard = dram.tile([cfg.shard_rows, d], f32, tag="h1shard")
            fu_tab = dram.tile([ubp, EPAD], bf, tag="futab")

            def spmm_region(sched, g, c, rl_t, rl_base, t_idx, table_ap,
                            vw_t, psums, first, last):
                """Emit gather/onehot/matmuls for one (group, chunk) region."""
                nidx = int(sched.region_nidx[g, c])
                if nidx == 0:
                    return
                rb = nidx // P
                w0 = int(sched.idx_off[g, c])
                it = idxp.tile([P, nidx // 16], mybir.dt.int16, tag="idx")
                nc.sync.dma_start(it[:], t_idx.ap()[:, w0 : w0 + nidx // 16])
                mb_t = mbp.tile([P, rb, d], bf, tag="msgsbf")
                for s0 in range(0, nidx, GSUB):
                    n = min(GSUB, nidx - s0)
                    _dma_gather128(
                        nc.gpsimd,
                        mb_t[:, s0 // P : (s0 + n) // P, :],
                        table_ap,
                        it[:, s0 // 16 : (s0 + n) // 16],
                        n, queue_num=next_q(),
                    )
                rboff = (int(sched.blk_base[g, c]) - rl_base)
                if vw_t is not None:
                    nc.vector.tensor_tensor(
                        out=mb_t[:],
                        in0=mb_t[:],
                        in1=vw_t[:, rboff : rboff + rb]
                        .unsqueeze(2)
                        .to_broadcast([P, rb, d]),
                        op=mybir.AluOpType.mult,
                    )
                oh_t = ohp.tile([P, rb, P], bf, tag="oh")
                nc.vector.tensor_tensor(
                    out=oh_t[:],
                    in0=rl_t[:, rboff : rboff + rb]
                    .unsqueeze(2)
                    .to_broadcast([P, rb, P]),
                    in1=iota_bf[:].unsqueeze(1).to_broadcast([P, rb, P]),
                    op=mybir.AluOpType.is_equal,
                )
                for j in range(rb):
                    gblk = int(sched.blk_base[g, c]) + j
                    # which tile does this block belong to?
                    soff = j * P
                    tl = 0
                    for tt in range(sched.tpg):
                        t_ = g * sched.tpg + tt
                        if (sched.sub_off[t_, c] <= soff
                                < sched.sub_off[t_, c] + sched.cap[t_, c]):
                            tl = tt
                            break
                    out_ap = psums[tl]
                    nc.tensor.matmul(
                        out_ap,
                        lhsT=mb_t[:, j, :],
                        rhs=oh_t[:, j, :],
                        start=(gblk == first[tl]),
                        stop=(gblk == last[tl]),
                    )

            def startstop(sched, g):
                first = {tl: sched.tile_blocks[(g, tl)][0]
                         for tl in range(sched.tpg) if sched.tile_blocks[(g, tl)]}
                last = {tl: sched.tile_blocks[(g, tl)][-1]
                        for tl in range(sched.tpg) if sched.tile_blocks[(g, tl)]}
                return first, last

            def spmm_group(sched, g, t_idx, t_rl, table_aps, val_t, psums):
                """Emit gathers/onehot/matmuls for one group (all chunks)."""
                gb0 = int(sched.group_blk0[g])
                gblocks = sched.group_blocks[g]
                if gblocks == 0:
                    return
                rl_t = rlp.tile([P, gblocks], bf, tag="rl")
                nc.sync.dma_start(rl_t[:], t_rl.ap()[:, gb0 : gb0 + gblocks])
                vw_t = None
                if val_t is not None:
                    vw_t = rlp.tile([P, gblocks], f32, tag="vw")
                    nc.sync.dma_start(vw_t[:], val_t.ap()[:, gb0 : gb0 + gblocks])
                first, last = startstop(sched, g)
                for c in range(sched.nch):
                    spmm_region(sched, g, c, rl_t, gb0, t_idx, table_aps[c],
                                vw_t, psums, first, last)

            def psum_packs(tpg):
                packs = []
                for i in range(0, tpg, 4):
                    w = min(4, tpg - i) * P
                    packs.append(
                        pap.tile([d, w], f32, tag="acc", name="accpk")
                    )
                return packs

            def tile_psum(packs, tl):
                return packs[tl // 4][:, (tl % 4) * P : (tl % 4 + 1) * P]

            # ================= L1 =================
            user_chunks = [
                t_userbf.ap()[c * cfg.ch_u : min((c + 1) * cfg.ch_u, cfg.n_user), 0:d]
                for c in range(cfg.nch_u)
            ]

            def issue_ag(k):
                nc.gpsimd.collective_compute(
                    "AllGather",
                    mybir.AluOpType.bypass,
                    replica_groups=[list(range(nc_))],
                    ins=[h1ag[k][:].opt()],
                    outs=[h1fb[k][:].opt()],
                )

            ag_issued = 0
            for g in range(s1.ng):
                packs = psum_packs(s1.tpg)
                psums = [tile_psum(packs, tl) for tl in range(s1.tpg)]
                spmm_group(
                    s1, g, t_l1i, t_l1r, user_chunks,
                    t_vals.get("l1"), psums,
                )
                u0_t = u0p.tile([P, s1.tpg, d], f32, tag="u0t")
                r0 = g * s1.tpg * P
                nc.sync.dma_start(
                    u0_t[:],
                    t_u0s.ap()[r0 : r0 + s1.tpg * P, :].rearrange(
                        "(t p) d -> p t d", p=P
                    ),
                )
                drains = []
                for pk in packs:
                    dsb = drp.tile([d, pk.shape[1]], f32, tag="drain")
                    nc.scalar.copy(dsb[:], pk[:])
                    drains.append(dsb)
                for tl in range(s1.tpg):
                    src = drains[tl // 4][:, (tl % 4) * P : (tl % 4 + 1) * P]
                    ptr = ptp.tile([P, d], f32, tag="tr")
                    nc.tensor.transpose(ptr[:], src, ident[:])
                    h1_t = hp.tile([P, d], f32, tag="hrow")
                    gt = g * s1.tpg + tl
                    nc.vector.scalar_tensor_tensor(
                        out=h1_t[:],
                        in0=ptr[:],
                        scalar=ist_t[:, gt : gt + 1],
                        in1=u0_t[:, tl, :],
                        op0=mybir.AluOpType.mult,
                        op1=mybir.AluOpType.add,
                    )
                    k = g // cfg.gpa
                    lrow = ((g % cfg.gpa) * s1.tpg + tl) * P
                    h1_b16 = hp.tile([P, EPAD], bf, tag="hrowb")
                    nc.scalar.copy(h1_b16[:, 0:d], h1_t[:])
                    nc.scalar.dma_start(
                        h1ag[k][lrow : lrow + P, :], h1_b16[:]
                    )
                    nc.scalar.dma_start(
                        h1_shard[gt * P : (gt + 1) * P, :], h1_t[:]
                    )
                # stagger AllGather issue 4 groups after its data is ready so
                # the Pool queue never stalls on the drain chain.
                if g >= 5 and (g - 5) % cfg.gpa == 0:
                    issue_ag((g - 5) // cfg.gpa)
                    ag_issued += 1

            # ================= INFO =================
            item_chunks = [
                t_itembf.ap()[c * cfg.ch_i : min((c + 1) * cfg.ch_i, cfg.n_item), 0:d]
                for c in range(cfg.nch_i)
            ]
            # run INFO first (independent of AllGather), then L2
            for g in range(si.ng):
                packs = psum_packs(si.tpg)
                psums = [tile_psum(packs, tl) for tl in range(si.tpg)]
                spmm_group(si, g, t_ini, t_inr, item_chunks,
                           t_vals.get("in"), psums)
                for ip, pk in enumerate(packs):
                    o0 = (g * si.tpg + ip * 4) * P
                    nc.scalar.copy(
                        t3T[:, o0 : o0 + pk.shape[1]], pk[:]
                    )

            # remaining AllGathers
            for k in range(ag_issued, cfg.agc):
                issue_ag(k)

            # h1 batch rows gather (from own shard)
            h1bi_t = pp.tile([P, ubp // 16], mybir.dt.int16, tag="h1bidx")
            nc.sync.dma_start(h1bi_t[:], t_h1bi.ap())
            for s0 in range(0, ubp, GSUB):
                n = min(GSUB, ubp - s0)
                nc.gpsimd.dma_gather(
                    h1b_t[:, s0 // P : (s0 + n) // P, :],
                    h1_shard[:],
                    h1bi_t[:, s0 // 16 : (s0 + n) // 16],
                    n, n, d, single_packet=False,
                    queue_num=next_q(),
                )

            # ================= L2 (chunk-major: AG chunk c feeds all groups
            # before chunk c+1, so a late AllGather never head-blocks) =====
            h1_chunks = [h1fb[k][:, 0:d] for k in range(cfg.agc)]
            rl2_t = pp.tile([P, s2.total_blocks], bf, tag="rl2")
            nc.sync.dma_start(rl2_t[:], t_l2r.ap())
            vw2_t = None
            if t_vals.get("l2") is not None:
                vw2_t = pp.tile([P, s2.total_blocks], f32, tag="vw2")
                nc.sync.dma_start(vw2_t[:], t_vals["l2"].ap())
            packs2 = [psum_packs(s2.tpg) for _ in range(s2.ng)]
            psums2 = [
                [tile_psum(packs2[g], tl) for tl in range(s2.tpg)]
                for g in range(s2.ng)
            ]
            ss2 = [startstop(s2, g) for g in range(s2.ng)]
            for c in range(s2.nch):
                for g in range(s2.ng):
                    spmm_region(s2, g, c, rl2_t, 0, t_l2i, h1_chunks[c],
                                vw2_t, psums2[g], ss2[g][0], ss2[g][1])
            for g in range(s2.ng):
                packs = packs2[g]
                drains = []
                for pk in packs:
                    dsb = drp.tile([d, pk.shape[1]], f32, tag="drain")
                    nc.scalar.copy(dsb[:], pk[:])
                    drains.append(dsb)
                for tl in range(s2.tpg):
                    T = g * s2.tpg + tl
                    src = drains[tl // 4][:, (tl % 4) * P : (tl % 4 + 1) * P]
                    ptr = ptp.tile([P, d], f32, tag="tr")
                    nc.tensor.transpose(ptr[:], src, ident[:])
                    x1 = hp.tile([P, d], f32, tag="hrow")
                    nc.vector.tensor_scalar_mul(
                        x1[:], ptr[:], isb_t[:, T : T + 1]
                    )
                    ptr3 = ptp.tile([P, d], f32, tag="tr")
                    nc.tensor.transpose(
                        ptr3[:], t3T[:, T * P : (T + 1) * P], ident[:]
                    )
                    x2 = hp.tile([P, d], f32, tag="hrow")
                    nc.vector.scalar_tensor_tensor(
                        out=x2[:], in0=ptr3[:],
                        scalar=iib_t[:, T : T + 1], in1=x1[:],
                        op0=mybir.AluOpType.mult, op1=mybir.AluOpType.add,
                    )
                    fu = hp.tile([P, d], f32, tag="hrow")
                    nc.vector.scalar_tensor_tensor(
                        out=fu[:], in0=h1b_t[:, T, :], scalar=2.0, in1=x2[:],
                        op0=mybir.AluOpType.mult, op1=mybir.AluOpType.add,
                    )
                    fub = hp.tile([P, EPAD], bf, tag="hrowb")
                    nc.scalar.copy(fub[:, 0:d], fu[:])
                    nc.scalar.dma_start(fu_tab[T * P : (T + 1) * P, :], fub[:])

            # ================= FINAL =================
            sc_t = pp.tile([P, ftot // P], f32, tag="scores")
            for chn in range(cfg.nch_i):
                n = int(fcap[chn])
                s0 = int(fbase[chn])
                fb = n // P
                iu = idxp.tile([P, n // 16], mybir.dt.int16, tag="idx")
                nc.sync.dma_start(
                    iu[:], t_pui.ap()[:, s0 // 16 : (s0 + n) // 16]
                )
                ii = idxp.tile([P, n // 16], mybir.dt.int16, tag="idx")
                nc.sync.dma_start(
                    ii[:], t_pii.ap()[:, s0 // 16 : (s0 + n) // 16]
                )
                u_t = mbp.tile([P, fb, d], bf, tag="msgsbf")
                v_t = mbp.tile([P, fb, d], bf, tag="msgsbf")
                for q0 in range(0, n, GSUB):
                    nq = min(GSUB, n - q0)
                    _dma_gather128(
                        nc.gpsimd,
                        u_t[:, q0 // P : (q0 + nq) // P, :], fu_tab[:, 0:d],
                        iu[:, q0 // 16 : (q0 + nq) // 16], nq,
                        queue_num=next_q(),
                    )
                    _dma_gather128(
                        nc.gpsimd,
                        v_t[:, q0 // P : (q0 + nq) // P, :], item_chunks[chn],
                        ii[:, q0 // 16 : (q0 + nq) // 16], nq,
                        queue_num=next_q(),
                    )
                pr = ohp.tile([P, fb, d], f32, tag="prod")
                nc.vector.tensor_mul(
                    pr[:], u_t[:], v_t[:]
                )
                dot = hp.tile([P, fb], f32, tag="dot")
                nc.vector.tensor_reduce(
                    dot[:], pr[:], axis=mybir.AxisListType.X,
                    op=mybir.AluOpType.add,
                )
                nc.scalar.activation(
                    sc_t[:, s0 // P : (s0 + n) // P], dot[:],
                    mybir.ActivationFunctionType.Sigmoid, scale=2.0,
                )
            nc.scalar.dma_start(t_scores.ap(), sc_t[:])

    nc.compile()
    return nc


_CACHE = {}


def _run(cfg, inputs, trace=False):
    import time as _time

    _t = _time.time()
    plan, in_maps, out_meta = _prep(cfg, inputs)
    print(f"[kernel] prep: {_time.time()-_t:.1f}s", flush=True)
    _t = _time.time()
    key = (
        cfg.n_user, plan["s1"].total_slots, plan["s2"].total_slots,
        plan["si"].total_slots, plan["ubt_p"], plan["ftot"], plan["ones"],
    )
    if key not in _CACHE:
        _CACHE[key] = _build_program(plan)
        print(f"[kernel] build+compile: {_time.time()-_t:.1f}s", flush=True)
    nc = _CACHE[key]
    _t = _time.time()
    kw = {}
    if trace:
        # single-core NTFF (SPMD cores are balanced); exec_time_ns comes back
        kw = dict(trace=True, trace_cores=[0])
    res = run_bass_kernel_spmd(
        nc, in_maps, core_ids=list(range(cfg.nc)), **kw
    )
    print(f"[kernel] run: {_time.time()-_t:.1f}s", flush=True)
    out = np.zeros(len(inputs["user_ids"]), np.float32)
    for c in range(cfg.nc):
        js, slots = out_meta[c]
        sc = res.results[c]["scores"]
        out[js] = sc[slots % P, slots // P]
    return out, res


def kernel(**inputs):
    out, _ = _run(REAL, inputs, trace=bool(os.environ.get("KERNEL_TRACE")))
    return out


# revision 23
# speedup vs baseline: 1.0055x; 1.0055x over previous
"""DiffNet GNN message-passing kernel for 8 Trainium2 NeuronCores.

Math: final_user = t2/deg_soc + 2*h1 + t3/deg_info, restricted to batch users,
where h1 = A_soc@u0/deg_soc + u0 (needed for ALL users since layer 2 gathers
arbitrary columns), t2 = A_soc@h1 at batch rows only, t3 = A_info@item_emb at
batch rows only. Output = sigmoid(2 * sum(final_user[uids] * item_emb[iids])).

Sharding: by output row range (12500 users/core). Layer-1 SpMM over the full
edge set partitions exactly by row; chunked AllGathers publish h1 (bf16,
128-wide padded rows so each gather element is 256B); layer-2 and info SpMMs
run only on each core's batch-user rows.

Device SpMM: edges sorted by (group, col-chunk, tile); dma_gather pulls 256B
message rows (padded bf16) from HBM tables directly into bf16 SBUF tiles;
SWDGE descriptor generation is spread across all 4 gpsimd queue core-pairs
(queue_num round-robin); a one-hot matrix (batched DVE is_equal) + PE matmul
(messages stationary [128e,64d], one-hot moving [128e,128r]) does the
segment-sum into PSUM; PE transpose + fused scale/residual produce rows.
"""

import sys

sys.path.insert(0, "/opt/trn_rl_repo")

import math
import os

import numpy as np
import ml_dtypes

import concourse.bacc as bacc
import concourse.bass as bass
import concourse.mybir as mybir
import concourse.tile as tile
from concourse.masks import make_identity
from concourse.bass_utils import run_bass_kernel_spmd

P = 128
BF16 = ml_dtypes.bfloat16
GSUB = 8192  # max idxs per dma_gather sub-call
EPAD = 128  # padded row width (bf16) -> 256B gather elements


class Cfg:
    def __init__(self, n_user, n_item, d, n_cores, tpg1, gpa, tpg2, chunk):
        self.n_user = n_user
        self.n_item = n_item
        self.d = d
        self.nc = n_cores
        self.rpc = n_user // n_cores  # rows per core
        t1 = -(-self.rpc // P)  # L1 tiles per core (unpadded)
        self.tpg1 = tpg1  # L1 tiles per group
        self.t1p = -(-t1 // tpg1) * tpg1  # padded L1 tile count
        self.ng1 = self.t1p // tpg1
        self.gpa = gpa  # groups per AllGather chunk
        assert self.ng1 % gpa == 0
        self.agc = self.ng1 // gpa  # number of AG chunks
        self.cr = tpg1 * P * gpa  # rows per core per AG chunk
        self.shard_rows = self.t1p * P
        self.flat_h1 = self.nc * self.shard_rows  # flat h1 rows
        self.tpg2 = tpg2  # batch tiles per group (L2 & info)
        self.chunk = chunk  # max gather-chunk rows (int16 limit)
        self.nch_u = -(-n_user // chunk)
        self.ch_u = -(-n_user // self.nch_u)
        self.nch_i = -(-n_item // chunk)
        self.ch_i = -(-n_item // self.nch_i)
        # L2 gathers directly from the per-AG-chunk h1 tables
        self.nch_h = self.agc
        self.ch_h = self.nc * self.cr
        assert self.ch_h < 2 ** 15


REAL = Cfg(100000, 50000, 64, 8, 7, 2, 4, 25088)


def _wrap_idx(idx_call):
    """[n] int16 -> [128, n/16] wrapped+replicated."""
    n = idx_call.shape[0]
    a = idx_call.reshape(n // 16, 16).T  # [16, n/16]
    return np.tile(a, (8, 1))


class SpmmSched:
    """SPMD-uniform slot/block layout for one SpMM (same across cores)."""

    def __init__(self, ntp, tpg, nch):
        self.ntp = ntp  # padded tile count
        self.tpg = tpg
        self.ng = ntp // tpg
        self.nch = nch
        self.cap = None  # [ntp, nch] slots, multiples of 128

    def finalize(self):
        ntp, tpg, ng, nch = self.ntp, self.tpg, self.ng, self.nch
        cap = self.cap
        # ensure every tile has >=1 block so its PSUM region gets zeroed
        for t in range(ntp):
            if cap[t].sum() == 0:
                cap[t, 0] = P
        # region = (g, c): tiles g*tpg..g*tpg+tpg-1
        self.sub_off = np.zeros((ntp, nch), np.int64)  # slot offset in region
        self.region_nidx = np.zeros((ng, nch), np.int64)
        self.slot_base = np.zeros((ng, nch), np.int64)  # global slot offset
        self.blk_base = np.zeros((ng, nch), np.int64)
        self.group_blk0 = np.zeros(ng, np.int64)
        s = 0
        b = 0
        for g in range(ng):
            self.group_blk0[g] = b
            for c in range(nch):
                self.slot_base[g, c] = s
                self.blk_base[g, c] = b
                off = 0
                for tl in range(tpg):
                    t = g * tpg + tl
                    self.sub_off[t, c] = off
                    off += cap[t, c]
                self.region_nidx[g, c] = off
                s += off
                b += off // P
        self.total_slots = s
        self.total_blocks = b
        self.group_blocks = [
            int(sum(self.region_nidx[g]) // P) for g in range(ng)
        ]
        # per (g, tl): ordered list of global block ids (for start/stop flags)
        self.tile_blocks = {}
        for g in range(ng):
            for tl in range(self.tpg):
                t = g * self.tpg + tl
                blks = []
                for c in range(nch):
                    b0 = self.blk_base[g, c] + self.sub_off[t, c] // P
                    blks += list(range(b0, b0 + cap[t, c] // P))
                self.tile_blocks[(g, tl)] = blks
        # idx array column offsets (global, in units of 16 slots)
        self.idx_off = np.zeros((ng, nch), np.int64)
        w = 0
        for g in range(ng):
            for c in range(nch):
                self.idx_off[g, c] = w
                w += self.region_nidx[g, c] // 16
        self.idx_w = w


def _sched_caps(sched, per_core_tc_counts):
    """per_core_tc_counts: list of [ntp, nch] arrays -> set caps."""
    mx = np.maximum.reduce(per_core_tc_counts)
    sched.cap = (-(-mx // P) * P).astype(np.int64)
    sched.finalize()


def _fill_spmm(sched, rows_t, cols_c, col_idx, rowloc, vals):
    """Place one core's edges into the schedule's slot space.

    rows_t: tile id per edge; cols_c: chunk id; col_idx: int16 local col;
    rowloc: row-in-tile (0..127); vals: edge values (float32).
    Returns (idx_arr [128, idx_w] i16, rl [128, B] bf16, val_w [128, B] f32).
    """
    ntp, tpg, ng, nch = sched.ntp, sched.tpg, sched.ng, sched.nch
    g_e = rows_t // tpg
    tl_e = rows_t % tpg
    bid = (g_e * nch + cols_c) * tpg + tl_e
    order = np.argsort(bid, kind="stable")
    bid_s = bid[order]
    counts = np.bincount(bid_s, minlength=ng * nch * tpg)
    starts = np.concatenate([[0], np.cumsum(counts)[:-1]])
    rank = np.arange(len(bid_s)) - starts[bid_s]
    t_s = rows_t[order]
    c_s = cols_c[order]
    g_s = g_e[order]
    slot = (
        sched.slot_base[g_s, c_s]
        + sched.sub_off[t_s, c_s]
        + rank
    )
    ns = sched.total_slots
    idx_flat = np.zeros(ns, np.int32)
    rl_flat = np.full(ns, -1.0, np.float32)
    val_flat = np.zeros(ns, np.float32)
    idx_flat[slot] = col_idx[order]
    rl_flat[slot] = rowloc[order]
    val_flat[slot] = vals[order]
    # pad slots keep idx 0 (real harmless gathers) so every slot is always
    # written -- avoids NaN garbage flowing into the matmul.
    idx_arr = np.empty((P, sched.idx_w), np.int16)
    for g in range(ng):
        for c in range(nch):
            n = sched.region_nidx[g, c]
            if n == 0:
                continue
            s0 = sched.slot_base[g, c]
            w0 = sched.idx_off[g, c]
            idx_arr[:, w0 : w0 + n // 16] = _wrap_idx(
                idx_flat[s0 : s0 + n].astype(np.int16)
            )
    rl = np.ascontiguousarray(
        rl_flat.reshape(sched.total_blocks, P).T.astype(BF16)
    )
    val_w = np.ascontiguousarray(val_flat.reshape(sched.total_blocks, P).T)
    return idx_arr, rl, val_w


def _prep(cfg, inputs):
    """All host-side preprocessing. Returns (plan, in_maps, out_meta)."""
    nc_, d = cfg.nc, cfg.d
    user_emb = np.asarray(inputs["user_emb"], np.float32)
    item_emb = np.asarray(inputs["item_emb"], np.float32)
    s_rows = np.asarray(inputs["social_rows"], np.int64)
    s_cols = np.asarray(inputs["social_cols"], np.int64)
    s_vals = np.asarray(inputs["social_vals"], np.float32)
    i_rows = np.asarray(inputs["info_rows"], np.int64)
    i_cols = np.asarray(inputs["info_cols"], np.int64)
    i_vals = np.asarray(inputs["info_vals"], np.float32)
    uids = np.asarray(inputs["user_ids"], np.int64)
    iids = np.asarray(inputs["item_ids"], np.int64)
    eps = 1e-8

    ones = bool(np.all(s_vals == 1.0) and np.all(i_vals == 1.0))

    deg_soc = np.bincount(s_rows, weights=s_vals, minlength=cfg.n_user)
    deg_info = np.bincount(i_rows, weights=i_vals, minlength=cfg.n_user)
    inv_soc = (1.0 / (deg_soc.astype(np.float32) + eps)).astype(np.float32)
    inv_info = (1.0 / (deg_info.astype(np.float32) + eps)).astype(np.float32)

    # padded bf16 gather tables (256B rows)
    user_bf = np.zeros((cfg.n_user, EPAD), BF16)
    user_bf[:, :d] = user_emb.astype(BF16)
    item_bf = np.zeros((cfg.n_item, EPAD), BF16)
    item_bf[:, :d] = item_emb.astype(BF16)

    # batch users
    uniq = np.unique(uids)
    owner = uniq // cfg.rpc
    bu = [uniq[owner == c] for c in range(nc_)]
    ubmax = max(len(b) for b in bu)
    ubt = -(-ubmax // P)
    ng2 = max(1, -(-ubt // cfg.tpg2))
    ubt_p = ng2 * cfg.tpg2
    ubp = ubt_p * P

    # --- L1 schedule ---
    s1 = SpmmSched(cfg.t1p, cfg.tpg1, cfg.nch_u)
    order = np.argsort(s_rows, kind="stable")
    sr, sc, sv = s_rows[order], s_cols[order], s_vals[order]
    bounds = np.searchsorted(sr, [c * cfg.rpc for c in range(nc_ + 1)])
    core_l1 = []
    tc_counts = []
    for c in range(nc_):
        lo, hi = bounds[c], bounds[c + 1]
        lr = sr[lo:hi] - c * cfg.rpc
        col = sc[lo:hi]
        t = lr // P
        ch = col // cfg.ch_u
        core_l1.append((t, ch, (col - ch * cfg.ch_u), lr % P, sv[lo:hi]))
        m = np.zeros((cfg.t1p, cfg.nch_u), np.int64)
        np.add.at(m, (t, ch), 1)
        tc_counts.append(m)
    _sched_caps(s1, tc_counts)

    # --- L2 & info: batch-row-restricted ---
    slot_of = np.full(cfg.n_user, -1, np.int64)
    for c in range(nc_):
        slot_of[bu[c]] = np.arange(len(bu[c]))
    in_batch = slot_of >= 0

    def batch_edges(rows, cols, vals):
        m = in_batch[rows]
        r, co, v = rows[m], cols[m], vals[m]
        core = r // cfg.rpc
        return r, co, v, core

    s2 = SpmmSched(ubt_p, cfg.tpg2, cfg.nch_h)
    si = SpmmSched(ubt_p, cfg.tpg2, cfg.nch_i)

    def h1_flat(col):
        own = col // cfg.rpc
        lr = col - own * cfg.rpc
        k = lr // cfg.cr
        off = lr - k * cfg.cr
        return k * (nc_ * cfg.cr) + own * cfg.cr + off

    r2, c2, v2, core2 = batch_edges(sr, sc, sv)
    f2 = h1_flat(c2)
    ri, ci, vi, corei = batch_edges(i_rows, i_cols, i_vals)

    core_l2, core_in = [], []
    tc2, tci = [], []
    for c in range(nc_):
        m = core2 == c
        sl = slot_of[r2[m]]
        t = sl // P
        ch = f2[m] // cfg.ch_h
        core_l2.append((t, ch, f2[m] - ch * cfg.ch_h, sl % P, v2[m]))
        a = np.zeros((ubt_p, cfg.nch_h), np.int64)
        np.add.at(a, (t, ch), 1)
        tc2.append(a)
        m = corei == c
        sl = slot_of[ri[m]]
        t = sl // P
        ch = ci[m] // cfg.ch_i
        core_in.append((t, ch, ci[m] - ch * cfg.ch_i, sl % P, vi[m]))
        a = np.zeros((ubt_p, cfg.nch_i), np.int64)
        np.add.at(a, (t, ch), 1)
        tci.append(a)
    _sched_caps(s2, tc2)
    _sched_caps(si, tci)

    # --- final pairs ---
    pcore = uids // cfg.rpc
    pch = iids // cfg.ch_i
    fcap = np.zeros(cfg.nch_i, np.int64)
    per_core_pairs = []
    for c in range(nc_):
        m = np.nonzero(pcore == c)[0]
        o = m[np.argsort(pch[m], kind="stable")]
        per_core_pairs.append(o)
        cnts = np.bincount(pch[o], minlength=cfg.nch_i)
        fcap = np.maximum(fcap, cnts)
    fcap = -(-fcap // P) * P
    fcap = np.maximum(fcap, P)
    fbase = np.concatenate([[0], np.cumsum(fcap)])
    ftot = int(fbase[-1])

    plan = dict(
        cfg=cfg, s1=s1, s2=s2, si=si, ubt_p=ubt_p, ubp=ubp, ng2=ng2,
        fcap=fcap, fbase=fbase, ftot=ftot, ones=ones,
    )

    in_maps = []
    out_meta = []  # per core: (pair_js, slots)
    for c in range(nc_):
        t, ch, cidx, rl, v = core_l1[c]
        l1_idx, l1_rl, l1_val = _fill_spmm(s1, t, ch, cidx, rl, v)
        t, ch, cidx, rl, v = core_l2[c]
        l2_idx, l2_rl, l2_val = _fill_spmm(s2, t, ch, cidx, rl, v)
        t, ch, cidx, rl, v = core_in[c]
        in_idx, in_rl, in_val = _fill_spmm(si, t, ch, cidx, rl, v)

        # u0 shard
        u0s = np.zeros((cfg.shard_rows, d), np.float32)
        nrow = min(cfg.rpc, cfg.n_user - c * cfg.rpc)
        u0s[:nrow] = user_emb[c * cfg.rpc : c * cfg.rpc + nrow]

        # invdeg arrays
        ist = np.zeros((P, cfg.t1p), np.float32)
        rows = c * cfg.rpc + np.arange(nrow)
        ist[np.arange(nrow) % P, np.arange(nrow) // P] = inv_soc[rows]
        isb = np.zeros((P, ubt_p), np.float32)
        iib = np.zeros((P, ubt_p), np.float32)
        nb = len(bu[c])
        isb[np.arange(nb) % P, np.arange(nb) // P] = inv_soc[bu[c]]
        iib[np.arange(nb) % P, np.arange(nb) // P] = inv_info[bu[c]]

        # h1 batch gather idx (local shard rows); pads gather row 0
        h1b = np.zeros(ubp, np.int16)
        h1b[:nb] = (bu[c] - c * cfg.rpc).astype(np.int16)

        # final pairs
        o = per_core_pairs[c]
        pu = np.zeros(ftot, np.int16)
        pi = np.zeros(ftot, np.int16)
        slots = np.empty(len(o), np.int64)
        pos = 0
        for chn in range(cfg.nch_i):
            sel = o[pch[o] == chn]
            k = len(sel)
            s0 = fbase[chn]
            pu[s0 : s0 + k] = slot_of[uids[sel]].astype(np.int16)
            pi[s0 : s0 + k] = (iids[sel] - chn * cfg.ch_i).astype(np.int16)
            slots[pos : pos + k] = s0 + np.arange(k)
            pos += k
        out_meta.append((o, slots))

        m = {
            "user_bf": user_bf,
            "item_bf": item_bf,
            "u0s": u0s,
            "l1_idx": l1_idx, "l1_rl": l1_rl,
            "l2_idx": l2_idx, "l2_rl": l2_rl,
            "in_idx": in_idx, "in_rl": in_rl,
            "ist": ist, "isb": isb, "iib": iib,
            "h1b_idx": _wrap_idx(h1b),
            "pu_idx": _wrap_idx(pu), "pi_idx": _wrap_idx(pi),
        }
        if not ones:
            m["l1_val"] = l1_val
            m["l2_val"] = l2_val
            m["in_val"] = in_val
        in_maps.append(m)
    return plan, in_maps, out_meta


def _build_program(plan):
    cfg = plan["cfg"]
    s1, s2, si = plan["s1"], plan["s2"], plan["si"]
    ubt_p, ubp, ng2 = plan["ubt_p"], plan["ubp"], plan["ng2"]
    fcap, fbase, ftot = plan["fcap"], plan["fbase"], plan["ftot"]
    ones = plan["ones"]
    d = cfg.d
    nc_ = cfg.nc
    f32 = mybir.dt.float32
    bf = mybir.dt.bfloat16

    # SWDGE descriptor carveout: max descs per DMA instruction is
    # dynamic_dma_scratch_size/16; gathers are split into GSUB-idx sub-calls.
    nc = bacc.Bacc("TRN2", debug=False, num_devices=nc_, num_swdge_queues=4)
    qrr = {"q": 0}

    def next_q():
        q = qrr["q"]
        qrr["q"] = (q + 1) % 4
        return q

    t_userbf = nc.dram_tensor("user_bf", [cfg.n_user, EPAD], bf, kind="ExternalInput")
    t_itembf = nc.dram_tensor("item_bf", [cfg.n_item, EPAD], bf, kind="ExternalInput")
    t_u0s = nc.dram_tensor("u0s", [cfg.shard_rows, d], f32, kind="ExternalInput")
    t_l1i = nc.dram_tensor("l1_idx", [P, s1.idx_w], mybir.dt.int16, kind="ExternalInput")
    t_l1r = nc.dram_tensor("l1_rl", [P, s1.total_blocks], bf, kind="ExternalInput")
    t_l2i = nc.dram_tensor("l2_idx", [P, s2.idx_w], mybir.dt.int16, kind="ExternalInput")
    t_l2r = nc.dram_tensor("l2_rl", [P, s2.total_blocks], bf, kind="ExternalInput")
    t_ini = nc.dram_tensor("in_idx", [P, si.idx_w], mybir.dt.int16, kind="ExternalInput")
    t_inr = nc.dram_tensor("in_rl", [P, si.total_blocks], bf, kind="ExternalInput")
    t_ist = nc.dram_tensor("ist", [P, cfg.t1p], f32, kind="ExternalInput")
    t_isb = nc.dram_tensor("isb", [P, ubt_p], f32, kind="ExternalInput")
    t_iib = nc.dram_tensor("iib", [P, ubt_p], f32, kind="ExternalInput")
    t_h1bi = nc.dram_tensor("h1b_idx", [P, ubp // 16], mybir.dt.int16, kind="ExternalInput")
    t_pui = nc.dram_tensor("pu_idx", [P, ftot // 16], mybir.dt.int16, kind="ExternalInput")
    t_pii = nc.dram_tensor("pi_idx", [P, ftot // 16], mybir.dt.int16, kind="ExternalInput")
    t_scores = nc.dram_tensor("scores", [P, ftot // P], f32, kind="ExternalOutput")
    t_vals = {}
    if not ones:
        t_vals["l1"] = nc.dram_tensor("l1_val", [P, s1.total_blocks], f32, kind="ExternalInput")
        t_vals["l2"] = nc.dram_tensor("l2_val", [P, s2.total_blocks], f32, kind="ExternalInput")
        t_vals["in"] = nc.dram_tensor("in_val", [P, si.total_blocks], f32, kind="ExternalInput")

    with tile.TileContext(nc) as tc:
        with (
            tc.tile_pool(name="const", bufs=1) as cp,
            tc.tile_pool(name="persist", bufs=1) as pp,
            tc.tile_pool(name="idx", bufs=5) as idxp,
            tc.tile_pool(name="msgsbf", bufs=6) as mbp,
            tc.tile_pool(name="oh", bufs=3) as ohp,
            tc.tile_pool(name="rl", bufs=4) as rlp,
            tc.tile_pool(name="u0t", bufs=4) as u0p,
            tc.tile_pool(name="drain", bufs=2) as drp,
            tc.tile_pool(name="hrow", bufs=4) as hp,
            tc.tile_pool(name="psacc", bufs=5, space="PSUM") as pap,
            tc.tile_pool(name="pstr", bufs=2, space="PSUM") as ptp,
            tc.tile_pool(name="dram", bufs=1, space="DRAM") as dram,
        ):
            # ---- constants / persistent ----
            iota_i = cp.tile([P, P], mybir.dt.int32, tag="iotai")
            nc.gpsimd.iota(iota_i[:], pattern=[[1, P]], base=0, channel_multiplier=0)
            iota_bf = cp.tile([P, P], bf, tag="iotabf")
            nc.vector.tensor_copy(iota_bf[:], iota_i[:])
            ident = cp.tile([d, d], f32, tag="ident")
            make_identity(nc, ident[:])
            ist_t = pp.tile([P, cfg.t1p], f32, tag="ist")
            nc.sync.dma_start(ist_t[:], t_ist.ap())
            isb_t = pp.tile([P, ubt_p], f32, tag="isb")
            nc.sync.dma_start(isb_t[:], t_isb.ap())
            iib_t = pp.tile([P, ubt_p], f32, tag="iib")
            nc.sync.dma_start(iib_t[:], t_iib.ap())
            t3T = pp.tile([d, ubp], f32, tag="t3T")
            h1b_t = pp.tile([P, ubt_p, d], f32, tag="h1b")
            nc.vector.memzero(h1b_t[:])

            # internal DRAM
            h1ag = [
                dram.tile([cfg.cr, EPAD], bf, tag=f"h1ag{k}", name=f"h1ag{k}")
                for k in range(cfg.agc)
            ]
            h1fb = [
                dram.tile([nc_ * cfg.cr, EPAD], bf, tag=f"h1fb{k}",
                          name=f"h1fb{k}")
                for k in range(cfg.agc)
            ]
            h1_shard = dram.tile([cfg.shard_rows, d], f32, tag="h1shard")
            fu_tab = dram.tile([ubp, EPAD], bf, tag="futab")

            def spmm_region(sched, g, c, rl_t, rl_base, t_idx, table_ap,
                            vw_t, psums, first, last):
                """Emit gather/onehot/matmuls for one (group, chunk) region."""
                nidx = int(sched.region_nidx[g, c])
                if nidx == 0:
                    return
                rb = nidx // P
                w0 = int(sched.idx_off[g, c])
                it = idxp.tile([P, nidx // 16], mybir.dt.int16, tag="idx")
                nc.sync.dma_start(it[:], t_idx.ap()[:, w0 : w0 + nidx // 16])
                mb_t = mbp.tile([P, rb, EPAD], bf, tag="msgsbf")
                for s0 in range(0, nidx, GSUB):
                    n = min(GSUB, nidx - s0)
                    nc.gpsimd.dma_gather(
                        mb_t[:, s0 // P : (s0 + n) // P, :],
                        table_ap,
                        it[:, s0 // 16 : (s0 + n) // 16],
                        n, n, EPAD, single_packet=False,
                        queue_num=next_q(),
                    )
                rboff = (int(sched.blk_base[g, c]) - rl_base)
                if vw_t is not None:
                    nc.vector.tensor_tensor(
                        out=mb_t[:, :, 0:d],
                        in0=mb_t[:, :, 0:d],
                        in1=vw_t[:, rboff : rboff + rb]
                        .unsqueeze(2)
                        .to_broadcast([P, rb, d]),
                        op=mybir.AluOpType.mult,
                    )
                oh_t = ohp.tile([P, rb, P], bf, tag="oh")
                nc.vector.tensor_tensor(
                    out=oh_t[:],
                    in0=rl_t[:, rboff : rboff + rb]
                    .unsqueeze(2)
                    .to_broadcast([P, rb, P]),
                    in1=iota_bf[:].unsqueeze(1).to_broadcast([P, rb, P]),
                    op=mybir.AluOpType.is_equal,
                )
                for j in range(rb):
                    gblk = int(sched.blk_base[g, c]) + j
                    # which tile does this block belong to?
                    soff = j * P
                    tl = 0
                    for tt in range(sched.tpg):
                        t_ = g * sched.tpg + tt
                        if (sched.sub_off[t_, c] <= soff
                                < sched.sub_off[t_, c] + sched.cap[t_, c]):
                            tl = tt
                            break
                    out_ap = psums[tl]
                    nc.tensor.matmul(
                        out_ap,
                        lhsT=mb_t[:, j, 0:d],
                        rhs=oh_t[:, j, :],
                        start=(gblk == first[tl]),
                        stop=(gblk == last[tl]),
                    )

            def startstop(sched, g):
                first = {tl: sched.tile_blocks[(g, tl)][0]
                         for tl in range(sched.tpg) if sched.tile_blocks[(g, tl)]}
                last = {tl: sched.tile_blocks[(g, tl)][-1]
                        for tl in range(sched.tpg) if sched.tile_blocks[(g, tl)]}
                return first, last

            def spmm_group(sched, g, t_idx, t_rl, table_aps, val_t, psums):
                """Emit gathers/onehot/matmuls for one group (all chunks)."""
                gb0 = int(sched.group_blk0[g])
                gblocks = sched.group_blocks[g]
                if gblocks == 0:
                    return
                rl_t = rlp.tile([P, gblocks], bf, tag="rl")
                nc.sync.dma_start(rl_t[:], t_rl.ap()[:, gb0 : gb0 + gblocks])
                vw_t = None
                if val_t is not None:
                    vw_t = rlp.tile([P, gblocks], f32, tag="vw")
                    nc.sync.dma_start(vw_t[:], val_t.ap()[:, gb0 : gb0 + gblocks])
                first, last = startstop(sched, g)
                for c in range(sched.nch):
                    spmm_region(sched, g, c, rl_t, gb0, t_idx, table_aps[c],
                                vw_t, psums, first, last)

            def psum_packs(tpg):
                packs = []
                for i in range(0, tpg, 4):
                    w = min(4, tpg - i) * P
                    packs.append(
                        pap.tile([d, w], f32, tag="acc", name="accpk")
                    )
                return packs

            def tile_psum(packs, tl):
                return packs[tl // 4][:, (tl % 4) * P : (tl % 4 + 1) * P]

            # ================= L1 =================
            user_chunks = [
                t_userbf.ap()[c * cfg.ch_u : min((c + 1) * cfg.ch_u, cfg.n_user), :]
                for c in range(cfg.nch_u)
            ]

            def issue_ag(k):
                nc.gpsimd.collective_compute(
                    "AllGather",
                    mybir.AluOpType.bypass,
                    replica_groups=[list(range(nc_))],
                    ins=[h1ag[k][:].opt()],
                    outs=[h1fb[k][:].opt()],
                )

            ag_issued = 0
            for g in range(s1.ng):
                packs = psum_packs(s1.tpg)
                psums = [tile_psum(packs, tl) for tl in range(s1.tpg)]
                spmm_group(
                    s1, g, t_l1i, t_l1r, user_chunks,
                    t_vals.get("l1"), psums,
                )
                u0_t = u0p.tile([P, s1.tpg, d], f32, tag="u0t")
                r0 = g * s1.tpg * P
                nc.sync.dma_start(
                    u0_t[:],
                    t_u0s.ap()[r0 : r0 + s1.tpg * P, :].rearrange(
                        "(t p) d -> p t d", p=P
                    ),
                )
                drains = []
                for pk in packs:
                    dsb = drp.tile([d, pk.shape[1]], f32, tag="drain")
                    nc.scalar.copy(dsb[:], pk[:])
                    drains.append(dsb)
                for tl in range(s1.tpg):
                    src = drains[tl // 4][:, (tl % 4) * P : (tl % 4 + 1) * P]
                    ptr = ptp.tile([P, d], f32, tag="tr")
                    nc.tensor.transpose(ptr[:], src, ident[:])
                    h1_t = hp.tile([P, d], f32, tag="hrow")
                    gt = g * s1.tpg + tl
                    nc.vector.scalar_tensor_tensor(
                        out=h1_t[:],
                        in0=ptr[:],
                        scalar=ist_t[:, gt : gt + 1],
                        in1=u0_t[:, tl, :],
                        op0=mybir.AluOpType.mult,
                        op1=mybir.AluOpType.add,
                    )
                    k = g // cfg.gpa
                    lrow = ((g % cfg.gpa) * s1.tpg + tl) * P
                    h1_b16 = hp.tile([P, EPAD], bf, tag="hrowb")
                    nc.scalar.copy(h1_b16[:, 0:d], h1_t[:])
                    nc.scalar.dma_start(
                        h1ag[k][lrow : lrow + P, :], h1_b16[:]
                    )
                    nc.scalar.dma_start(
                        h1_shard[gt * P : (gt + 1) * P, :], h1_t[:]
                    )
                # stagger AllGather issue 4 groups after its data is ready so
                # the Pool queue never stalls on the drain chain.
                if g >= 5 and (g - 5) % cfg.gpa == 0:
                    issue_ag((g - 5) // cfg.gpa)
                    ag_issued += 1

            # ================= INFO =================
            item_chunks = [
                t_itembf.ap()[c * cfg.ch_i : min((c + 1) * cfg.ch_i, cfg.n_item), :]
                for c in range(cfg.nch_i)
            ]
            # run INFO first (independent of AllGather), then L2
            for g in range(si.ng):
                packs = psum_packs(si.tpg)
                psums = [tile_psum(packs, tl) for tl in range(si.tpg)]
                spmm_group(si, g, t_ini, t_inr, item_chunks,
                           t_vals.get("in"), psums)
                for ip, pk in enumerate(packs):
                    o0 = (g * si.tpg + ip * 4) * P
                    nc.scalar.copy(
                        t3T[:, o0 : o0 + pk.shape[1]], pk[:]
                    )

            # remaining AllGathers
            for k in range(ag_issued, cfg.agc):
                issue_ag(k)

            # h1 batch rows gather (from own shard)
            h1bi_t = pp.tile([P, ubp // 16], mybir.dt.int16, tag="h1bidx")
            nc.sync.dma_start(h1bi_t[:], t_h1bi.ap())
            for s0 in range(0, ubp, GSUB):
                n = min(GSUB, ubp - s0)
                nc.gpsimd.dma_gather(
                    h1b_t[:, s0 // P : (s0 + n) // P, :],
                    h1_shard[:],
                    h1bi_t[:, s0 // 16 : (s0 + n) // 16],
                    n, n, d, single_packet=False,
                    queue_num=next_q(),
                )

            # ================= L2 (chunk-major) =================
            h1_chunks = [h1fb[k][:] for k in range(cfg.agc)]
            rl2_t = pp.tile([P, s2.total_blocks], bf, tag="rl2")
            nc.sync.dma_start(rl2_t[:], t_l2r.ap())
            vw2_t = None
            if t_vals.get("l2") is not None:
                vw2_t = pp.tile([P, s2.total_blocks], f32, tag="vw2")
                nc.sync.dma_start(vw2_t[:], t_vals["l2"].ap())
            packs2 = [psum_packs(s2.tpg) for _ in range(s2.ng)]
            psums2 = [
                [tile_psum(packs2[g], tl) for tl in range(s2.tpg)]
                for g in range(s2.ng)
            ]
            ss2 = [startstop(s2, g) for g in range(s2.ng)]
            for c in range(s2.nch):
                for g in range(s2.ng):
                    spmm_region(s2, g, c, rl2_t, 0, t_l2i, h1_chunks[c],
                                vw2_t, psums2[g], ss2[g][0], ss2[g][1])
            for g in range(s2.ng):
                packs = packs2[g]
                drains = []
                for pk in packs:
                    dsb = drp.tile([d, pk.shape[1]], f32, tag="drain")
                    nc.scalar.copy(dsb[:], pk[:])
                    drains.append(dsb)
                for tl in range(s2.tpg):
                    T = g * s2.tpg + tl
                    src = drains[tl // 4][:, (tl % 4) * P : (tl % 4 + 1) * P]
                    ptr = ptp.tile([P, d], f32, tag="tr")
                    nc.tensor.transpose(ptr[:], src, ident[:])
                    x1 = hp.tile([P, d], f32, tag="hrow")
                    nc.vector.tensor_scalar_mul(
                        x1[:], ptr[:], isb_t[:, T : T + 1]
                    )
                    ptr3 = ptp.tile([P, d], f32, tag="tr")
                    nc.tensor.transpose(
                        ptr3[:], t3T[:, T * P : (T + 1) * P], ident[:]
                    )
                    x2 = hp.tile([P, d], f32, tag="hrow")
                    nc.vector.scalar_tensor_tensor(
                        out=x2[:], in0=ptr3[:],
                        scalar=iib_t[:, T : T + 1], in1=x1[:],
                        op0=mybir.AluOpType.mult, op1=mybir.AluOpType.add,
                    )
                    fu = hp.tile([P, d], f32, tag="hrow")
                    nc.vector.scalar_tensor_tensor(
                        out=fu[:], in0=h1b_t[:, T, :], scalar=2.0, in1=x2[:],
                        op0=mybir.AluOpType.mult, op1=mybir.AluOpType.add,
                    )
                    fub = hp.tile([P, EPAD], bf, tag="hrowb")
                    nc.scalar.copy(fub[:, 0:d], fu[:])
                    nc.scalar.dma_start(fu_tab[T * P : (T + 1) * P, :], fub[:])

            # ================= FINAL =================
            sc_t = pp.tile([P, ftot // P], f32, tag="scores")
            for chn in range(cfg.nch_i):
                n = int(fcap[chn])
                s0 = int(fbase[chn])
                fb = n // P
                iu = idxp.tile([P, n // 16], mybir.dt.int16, tag="idx")
                nc.sync.dma_start(
                    iu[:], t_pui.ap()[:, s0 // 16 : (s0 + n) // 16]
                )
                ii = idxp.tile([P, n // 16], mybir.dt.int16, tag="idx")
                nc.sync.dma_start(
                    ii[:], t_pii.ap()[:, s0 // 16 : (s0 + n) // 16]
                )
                u_t = mbp.tile([P, fb, EPAD], bf, tag="msgsbf")
                v_t = mbp.tile([P, fb, EPAD], bf, tag="msgsbf")
                for q0 in range(0, n, GSUB):
                    nq = min(GSUB, n - q0)
                    nc.gpsimd.dma_gather(
                        u_t[:, q0 // P : (q0 + nq) // P, :], fu_tab[:],
                        iu[:, q0 // 16 : (q0 + nq) // 16], nq, nq, EPAD,
                        single_packet=False, queue_num=next_q(),
                    )
                    nc.gpsimd.dma_gather(
                        v_t[:, q0 // P : (q0 + nq) // P, :], item_chunks[chn],
                        ii[:, q0 // 16 : (q0 + nq) // 16], nq, nq, EPAD,
                        single_packet=False, queue_num=next_q(),
                    )
                pr = ohp.tile([P, fb, d], f32, tag="prod")
                nc.vector.tensor_mul(
                    pr[:], u_t[:, :, 0:d], v_t[:, :, 0:d]
                )
                dot = hp.tile([P, fb], f32, tag="dot")
                nc.vector.tensor_reduce(
                    dot[:], pr[:], axis=mybir.AxisListType.X,
                    op=mybir.AluOpType.add,
                )
                nc.scalar.activation(
                    sc_t[:, s0 // P : (s0 + n) // P], dot[:],
                    mybir.ActivationFunctionType.Sigmoid, scale=2.0,
                )
            nc.scalar.dma_start(t_scores.ap(), sc_t[:])

    nc.compile()
    return nc


_CACHE = {}


def _run(cfg, inputs, trace=False):
    import time as _time

    _t = _time.time()
    plan, in_maps, out_meta = _prep(cfg, inputs)
    print(f"[kernel] prep: {_time.time()-_t:.1f}s", flush=True)
    _t = _time.time()
    key = (
        cfg.n_user, plan["s1"].total_slots, plan["s2"].total_slots,
        plan["si"].total_slots, plan["ubt_p"], plan["ftot"], plan["ones"],
    )
    if key not in _CACHE:
        _CACHE[key] = _build_program(plan)
        print(f"[kernel] build+compile: {_time.time()-_t:.1f}s", flush=True)
    nc = _CACHE[key]
    _t = _time.time()
    kw = {}
    if trace:
        # single-core NTFF (SPMD cores are balanced); exec_time_ns comes back
        kw = dict(trace=True, trace_cores=[0])
    res = run_bass_kernel_spmd(
        nc, in_maps, core_ids=list(range(cfg.nc)), **kw
    )
    print(f"[kernel] run: {_time.time()-_t:.1f}s", flush=True)
    out = np.zeros(len(inputs["user_ids"]), np.float32)
    for c in range(cfg.nc):
        js, slots = out_meta[c]
        sc = res.results[c]["scores"]
        out[js] = sc[slots % P, slots // P]
    return out, res


def kernel(**inputs):
    out, _ = _run(REAL, inputs, trace=bool(os.environ.get("KERNEL_TRACE")))
    return out


# revision 24
# speedup vs baseline: 1.3421x; 1.3347x over previous
"""DiffNet GNN message-passing kernel for 8 Trainium2 NeuronCores.

Math: final_user = t2/deg_soc + 2*h1 + t3/deg_info, restricted to batch users,
where h1 = A_soc@u0/deg_soc + u0 (needed for ALL users since layer 2 gathers
arbitrary columns), t2 = A_soc@h1 at batch rows only, t3 = A_info@item_emb at
batch rows only. Output = sigmoid(2 * sum(final_user[uids] * item_emb[iids])).

Sharding: by output row range (12500 users/core). Layer-1 SpMM over the full
edge set partitions exactly by row; chunked AllGathers publish h1 (bf16,
128-wide padded rows so each gather element is 256B); layer-2 and info SpMMs
run only on each core's batch-user rows.

Device SpMM: edges sorted by (group, col-chunk, tile); dma_gather pulls 256B
message rows (padded bf16) from HBM tables directly into bf16 SBUF tiles;
SWDGE descriptor generation is spread across all 4 gpsimd queue core-pairs
(queue_num round-robin); a one-hot matrix (batched DVE is_equal) + PE matmul
(messages stationary [128e,64d], one-hot moving [128e,128r]) does the
segment-sum into PSUM; PE transpose + fused scale/residual produce rows.
"""

import sys

sys.path.insert(0, "/opt/trn_rl_repo")

import math
import os

import numpy as np
import ml_dtypes

import concourse.bacc as bacc
import concourse.bass as bass
import concourse.mybir as mybir
import concourse.tile as tile
from concourse.masks import make_identity
from concourse.bass_utils import run_bass_kernel_spmd

P = 128
BF16 = ml_dtypes.bfloat16
GSUB = 8192  # max idxs per dma_gather sub-call
EPAD = 128  # padded row width (bf16) -> 256B gather elements


class Cfg:
    def __init__(self, n_user, n_item, d, n_cores, tpg1, gpa, tpg2, chunk):
        self.n_user = n_user
        self.n_item = n_item
        self.d = d
        self.nc = n_cores
        self.rpc = n_user // n_cores  # rows per core
        t1 = -(-self.rpc // P)  # L1 tiles per core (unpadded)
        self.tpg1 = tpg1  # L1 tiles per group
        self.t1p = -(-t1 // tpg1) * tpg1  # padded L1 tile count
        self.ng1 = self.t1p // tpg1
        self.gpa = gpa  # groups per AllGather chunk
        assert self.ng1 % gpa == 0
        self.agc = self.ng1 // gpa  # number of AG chunks
        self.cr = tpg1 * P * gpa  # rows per core per AG chunk
        self.shard_rows = self.t1p * P
        self.flat_h1 = self.nc * self.shard_rows  # flat h1 rows
        self.tpg2 = tpg2  # batch tiles per group (L2 & info)
        self.chunk = chunk  # max gather-chunk rows (int16 limit)
        self.nch_u = -(-n_user // chunk)
        self.ch_u = -(-n_user // self.nch_u)
        self.nch_i = -(-n_item // chunk)
        self.ch_i = -(-n_item // self.nch_i)
        # L2 gathers directly from the per-AG-chunk h1 tables
        self.nch_h = self.agc
        self.ch_h = self.nc * self.cr
        assert self.ch_h < 2 ** 15


REAL = Cfg(100000, 50000, 64, 8, 2, 7, 4, 25088)


def _wrap_idx(idx_call):
    """[n] int16 -> [128, n/16] wrapped+replicated."""
    n = idx_call.shape[0]
    a = idx_call.reshape(n // 16, 16).T  # [16, n/16]
    return np.tile(a, (8, 1))


class SpmmSched:
    """SPMD-uniform slot/block layout for one SpMM (same across cores)."""

    def __init__(self, ntp, tpg, nch):
        self.ntp = ntp  # padded tile count
        self.tpg = tpg
        self.ng = ntp // tpg
        self.nch = nch
        self.cap = None  # [ntp, nch] slots, multiples of 128

    def finalize(self):
        ntp, tpg, ng, nch = self.ntp, self.tpg, self.ng, self.nch
        cap = self.cap
        # ensure every tile has >=1 block so its PSUM region gets zeroed
        for t in range(ntp):
            if cap[t].sum() == 0:
                cap[t, 0] = P
        # region = (g, c): tiles g*tpg..g*tpg+tpg-1
        self.sub_off = np.zeros((ntp, nch), np.int64)  # slot offset in region
        self.region_nidx = np.zeros((ng, nch), np.int64)
        self.slot_base = np.zeros((ng, nch), np.int64)  # global slot offset
        self.blk_base = np.zeros((ng, nch), np.int64)
        self.group_blk0 = np.zeros(ng, np.int64)
        s = 0
        b = 0
        for g in range(ng):
            self.group_blk0[g] = b
            for c in range(nch):
                self.slot_base[g, c] = s
                self.blk_base[g, c] = b
                off = 0
                for tl in range(tpg):
                    t = g * tpg + tl
                    self.sub_off[t, c] = off
                    off += cap[t, c]
                self.region_nidx[g, c] = off
                s += off
                b += off // P
        self.total_slots = s
        self.total_blocks = b
        self.group_blocks = [
            int(sum(self.region_nidx[g]) // P) for g in range(ng)
        ]
        # per (g, tl): ordered list of global block ids (for start/stop flags)
        self.tile_blocks = {}
        for g in range(ng):
            for tl in range(self.tpg):
                t = g * self.tpg + tl
                blks = []
                for c in range(nch):
                    b0 = self.blk_base[g, c] + self.sub_off[t, c] // P
                    blks += list(range(b0, b0 + cap[t, c] // P))
                self.tile_blocks[(g, tl)] = blks
        # idx array column offsets (global, in units of 16 slots)
        self.idx_off = np.zeros((ng, nch), np.int64)
        w = 0
        for g in range(ng):
            for c in range(nch):
                self.idx_off[g, c] = w
                w += self.region_nidx[g, c] // 16
        self.idx_w = w


def _sched_caps(sched, per_core_tc_counts):
    """per_core_tc_counts: list of [ntp, nch] arrays -> set caps."""
    mx = np.maximum.reduce(per_core_tc_counts)
    sched.cap = (-(-mx // P) * P).astype(np.int64)
    sched.finalize()


def _fill_spmm(sched, rows_t, cols_c, col_idx, rowloc, vals):
    """Place one core's edges into the schedule's slot space.

    rows_t: tile id per edge; cols_c: chunk id; col_idx: int16 local col;
    rowloc: row-in-tile (0..127); vals: edge values (float32).
    Returns (idx_arr [128, idx_w] i16, rl [128, B] bf16, val_w [128, B] f32).
    """
    ntp, tpg, ng, nch = sched.ntp, sched.tpg, sched.ng, sched.nch
    g_e = rows_t // tpg
    tl_e = rows_t % tpg
    bid = (g_e * nch + cols_c) * tpg + tl_e
    order = np.argsort(bid, kind="stable")
    bid_s = bid[order]
    counts = np.bincount(bid_s, minlength=ng * nch * tpg)
    starts = np.concatenate([[0], np.cumsum(counts)[:-1]])
    rank = np.arange(len(bid_s)) - starts[bid_s]
    t_s = rows_t[order]
    c_s = cols_c[order]
    g_s = g_e[order]
    slot = (
        sched.slot_base[g_s, c_s]
        + sched.sub_off[t_s, c_s]
        + rank
    )
    ns = sched.total_slots
    idx_flat = np.zeros(ns, np.int32)
    rl_flat = np.full(ns, -1.0, np.float32)
    val_flat = np.zeros(ns, np.float32)
    idx_flat[slot] = col_idx[order]
    rl_flat[slot] = rowloc[order]
    val_flat[slot] = vals[order]
    # pad slots keep idx 0 (real harmless gathers) so every slot is always
    # written -- avoids NaN garbage flowing into the matmul.
    idx_arr = np.empty((P, sched.idx_w), np.int16)
    for g in range(ng):
        for c in range(nch):
            n = sched.region_nidx[g, c]
            if n == 0:
                continue
            s0 = sched.slot_base[g, c]
            w0 = sched.idx_off[g, c]
            idx_arr[:, w0 : w0 + n // 16] = _wrap_idx(
                idx_flat[s0 : s0 + n].astype(np.int16)
            )
    rl = np.ascontiguousarray(
        rl_flat.reshape(sched.total_blocks, P).T.astype(BF16)
    )
    val_w = np.ascontiguousarray(val_flat.reshape(sched.total_blocks, P).T)
    return idx_arr, rl, val_w


def _prep(cfg, inputs):
    """All host-side preprocessing. Returns (plan, in_maps, out_meta)."""
    nc_, d = cfg.nc, cfg.d
    user_emb = np.asarray(inputs["user_emb"], np.float32)
    item_emb = np.asarray(inputs["item_emb"], np.float32)
    s_rows = np.asarray(inputs["social_rows"], np.int64)
    s_cols = np.asarray(inputs["social_cols"], np.int64)
    s_vals = np.asarray(inputs["social_vals"], np.float32)
    i_rows = np.asarray(inputs["info_rows"], np.int64)
    i_cols = np.asarray(inputs["info_cols"], np.int64)
    i_vals = np.asarray(inputs["info_vals"], np.float32)
    uids = np.asarray(inputs["user_ids"], np.int64)
    iids = np.asarray(inputs["item_ids"], np.int64)
    eps = 1e-8

    ones = bool(np.all(s_vals == 1.0) and np.all(i_vals == 1.0))

    deg_soc = np.bincount(s_rows, weights=s_vals, minlength=cfg.n_user)
    deg_info = np.bincount(i_rows, weights=i_vals, minlength=cfg.n_user)
    inv_soc = (1.0 / (deg_soc.astype(np.float32) + eps)).astype(np.float32)
    inv_info = (1.0 / (deg_info.astype(np.float32) + eps)).astype(np.float32)

    # padded bf16 gather tables (256B rows)
    user_bf = np.zeros((cfg.n_user, EPAD), BF16)
    user_bf[:, :d] = user_emb.astype(BF16)
    item_bf = np.zeros((cfg.n_item, EPAD), BF16)
    item_bf[:, :d] = item_emb.astype(BF16)

    # batch users
    uniq = np.unique(uids)
    owner = uniq // cfg.rpc
    bu = [uniq[owner == c] for c in range(nc_)]
    ubmax = max(len(b) for b in bu)
    ubt = -(-ubmax // P)
    ng2 = max(1, -(-ubt // cfg.tpg2))
    ubt_p = ng2 * cfg.tpg2
    ubp = ubt_p * P

    # --- L1 schedule ---
    s1 = SpmmSched(cfg.t1p, cfg.tpg1, cfg.nch_u)
    order = np.argsort(s_rows, kind="stable")
    sr, sc, sv = s_rows[order], s_cols[order], s_vals[order]
    bounds = np.searchsorted(sr, [c * cfg.rpc for c in range(nc_ + 1)])
    core_l1 = []
    tc_counts = []
    for c in range(nc_):
        lo, hi = bounds[c], bounds[c + 1]
        lr = sr[lo:hi] - c * cfg.rpc
        col = sc[lo:hi]
        t = lr // P
        ch = col // cfg.ch_u
        core_l1.append((t, ch, (col - ch * cfg.ch_u), lr % P, sv[lo:hi]))
        m = np.zeros((cfg.t1p, cfg.nch_u), np.int64)
        np.add.at(m, (t, ch), 1)
        tc_counts.append(m)
    _sched_caps(s1, tc_counts)

    # --- L2 & info: batch-row-restricted ---
    slot_of = np.full(cfg.n_user, -1, np.int64)
    for c in range(nc_):
        slot_of[bu[c]] = np.arange(len(bu[c]))
    in_batch = slot_of >= 0

    def batch_edges(rows, cols, vals):
        m = in_batch[rows]
        r, co, v = rows[m], cols[m], vals[m]
        core = r // cfg.rpc
        return r, co, v, core

    s2 = SpmmSched(ubt_p, cfg.tpg2, cfg.nch_h)
    si = SpmmSched(ubt_p, 1, cfg.nch_i)

    def h1_flat(col):
        own = col // cfg.rpc
        lr = col - own * cfg.rpc
        k = lr // cfg.cr
        off = lr - k * cfg.cr
        return k * (nc_ * cfg.cr) + own * cfg.cr + off

    r2, c2, v2, core2 = batch_edges(sr, sc, sv)
    f2 = h1_flat(c2)
    ri, ci, vi, corei = batch_edges(i_rows, i_cols, i_vals)

    core_l2, core_in = [], []
    tc2, tci = [], []
    for c in range(nc_):
        m = core2 == c
        sl = slot_of[r2[m]]
        t = sl // P
        ch = f2[m] // cfg.ch_h
        core_l2.append((t, ch, f2[m] - ch * cfg.ch_h, sl % P, v2[m]))
        a = np.zeros((ubt_p, cfg.nch_h), np.int64)
        np.add.at(a, (t, ch), 1)
        tc2.append(a)
        m = corei == c
        sl = slot_of[ri[m]]
        t = sl // P
        ch = ci[m] // cfg.ch_i
        core_in.append((t, ch, ci[m] - ch * cfg.ch_i, sl % P, vi[m]))
        a = np.zeros((ubt_p, cfg.nch_i), np.int64)
        np.add.at(a, (t, ch), 1)
        tci.append(a)
    _sched_caps(s2, tc2)
    _sched_caps(si, tci)

    # --- final pairs ---
    pcore = uids // cfg.rpc
    pch = iids // cfg.ch_i
    fcap = np.zeros(cfg.nch_i, np.int64)
    per_core_pairs = []
    for c in range(nc_):
        m = np.nonzero(pcore == c)[0]
        o = m[np.argsort(pch[m], kind="stable")]
        per_core_pairs.append(o)
        cnts = np.bincount(pch[o], minlength=cfg.nch_i)
        fcap = np.maximum(fcap, cnts)
    fcap = -(-fcap // P) * P
    fcap = np.maximum(fcap, P)
    fbase = np.concatenate([[0], np.cumsum(fcap)])
    ftot = int(fbase[-1])

    plan = dict(
        cfg=cfg, s1=s1, s2=s2, si=si, ubt_p=ubt_p, ubp=ubp, ng2=ng2,
        fcap=fcap, fbase=fbase, ftot=ftot, ones=ones,
    )

    in_maps = []
    out_meta = []  # per core: (pair_js, slots)
    for c in range(nc_):
        t, ch, cidx, rl, v = core_l1[c]
        l1_idx, l1_rl, l1_val = _fill_spmm(s1, t, ch, cidx, rl, v)
        t, ch, cidx, rl, v = core_l2[c]
        l2_idx, l2_rl, l2_val = _fill_spmm(s2, t, ch, cidx, rl, v)
        t, ch, cidx, rl, v = core_in[c]
        in_idx, in_rl, in_val = _fill_spmm(si, t, ch, cidx, rl, v)

        # u0 shard
        u0s = np.zeros((cfg.shard_rows, d), np.float32)
        nrow = min(cfg.rpc, cfg.n_user - c * cfg.rpc)
        u0s[:nrow] = user_emb[c * cfg.rpc : c * cfg.rpc + nrow]

        # invdeg arrays
        ist = np.zeros((P, cfg.t1p), np.float32)
        rows = c * cfg.rpc + np.arange(nrow)
        ist[np.arange(nrow) % P, np.arange(nrow) // P] = inv_soc[rows]
        isb = np.zeros((P, ubt_p), np.float32)
        iib = np.zeros((P, ubt_p), np.float32)
        nb = len(bu[c])
        isb[np.arange(nb) % P, np.arange(nb) // P] = inv_soc[bu[c]]
        iib[np.arange(nb) % P, np.arange(nb) // P] = inv_info[bu[c]]

        # h1 batch gather idx (local shard rows); pads gather row 0
        h1b = np.zeros(ubp, np.int16)
        h1b[:nb] = (bu[c] - c * cfg.rpc).astype(np.int16)

        # final pairs
        o = per_core_pairs[c]
        pu = np.zeros(ftot, np.int16)
        pi = np.zeros(ftot, np.int16)
        slots = np.empty(len(o), np.int64)
        pos = 0
        for chn in range(cfg.nch_i):
            sel = o[pch[o] == chn]
            k = len(sel)
            s0 = fbase[chn]
            pu[s0 : s0 + k] = slot_of[uids[sel]].astype(np.int16)
            pi[s0 : s0 + k] = (iids[sel] - chn * cfg.ch_i).astype(np.int16)
            slots[pos : pos + k] = s0 + np.arange(k)
            pos += k
        out_meta.append((o, slots))

        m = {
            "user_bf": user_bf,
            "item_bf": item_bf,
            "u0s": u0s,
            "l1_idx": l1_idx, "l1_rl": l1_rl,
            "l2_idx": l2_idx, "l2_rl": l2_rl,
            "in_idx": in_idx, "in_rl": in_rl,
            "ist": ist, "isb": isb, "iib": iib,
            "h1b_idx": _wrap_idx(h1b),
            "pu_idx": _wrap_idx(pu), "pi_idx": _wrap_idx(pi),
        }
        if not ones:
            m["l1_val"] = l1_val
            m["l2_val"] = l2_val
            m["in_val"] = in_val
        in_maps.append(m)
    return plan, in_maps, out_meta


def _build_program(plan):
    cfg = plan["cfg"]
    s1, s2, si = plan["s1"], plan["s2"], plan["si"]
    ubt_p, ubp, ng2 = plan["ubt_p"], plan["ubp"], plan["ng2"]
    fcap, fbase, ftot = plan["fcap"], plan["fbase"], plan["ftot"]
    ones = plan["ones"]
    d = cfg.d
    nc_ = cfg.nc
    f32 = mybir.dt.float32
    bf = mybir.dt.bfloat16

    # SWDGE descriptor carveout: max descs per DMA instruction is
    # dynamic_dma_scratch_size/16; gathers are split into GSUB-idx sub-calls.
    nc = bacc.Bacc("TRN2", debug=False, num_devices=nc_, num_swdge_queues=4)
    qrr = {"q": 0}

    def next_q():
        q = qrr["q"]
        qrr["q"] = (q + 1) % 4
        return q

    t_userbf = nc.dram_tensor("user_bf", [cfg.n_user, EPAD], bf, kind="ExternalInput")
    t_itembf = nc.dram_tensor("item_bf", [cfg.n_item, EPAD], bf, kind="ExternalInput")
    t_u0s = nc.dram_tensor("u0s", [cfg.shard_rows, d], f32, kind="ExternalInput")
    t_l1i = nc.dram_tensor("l1_idx", [P, s1.idx_w], mybir.dt.int16, kind="ExternalInput")
    t_l1r = nc.dram_tensor("l1_rl", [P, s1.total_blocks], bf, kind="ExternalInput")
    t_l2i = nc.dram_tensor("l2_idx", [P, s2.idx_w], mybir.dt.int16, kind="ExternalInput")
    t_l2r = nc.dram_tensor("l2_rl", [P, s2.total_blocks], bf, kind="ExternalInput")
    t_ini = nc.dram_tensor("in_idx", [P, si.idx_w], mybir.dt.int16, kind="ExternalInput")
    t_inr = nc.dram_tensor("in_rl", [P, si.total_blocks], bf, kind="ExternalInput")
    t_ist = nc.dram_tensor("ist", [P, cfg.t1p], f32, kind="ExternalInput")
    t_isb = nc.dram_tensor("isb", [P, ubt_p], f32, kind="ExternalInput")
    t_iib = nc.dram_tensor("iib", [P, ubt_p], f32, kind="ExternalInput")
    t_h1bi = nc.dram_tensor("h1b_idx", [P, ubp // 16], mybir.dt.int16, kind="ExternalInput")
    t_pui = nc.dram_tensor("pu_idx", [P, ftot // 16], mybir.dt.int16, kind="ExternalInput")
    t_pii = nc.dram_tensor("pi_idx", [P, ftot // 16], mybir.dt.int16, kind="ExternalInput")
    t_scores = nc.dram_tensor("scores", [P, ftot // P], f32, kind="ExternalOutput")
    t_vals = {}
    if not ones:
        t_vals["l1"] = nc.dram_tensor("l1_val", [P, s1.total_blocks], f32, kind="ExternalInput")
        t_vals["l2"] = nc.dram_tensor("l2_val", [P, s2.total_blocks], f32, kind="ExternalInput")
        t_vals["in"] = nc.dram_tensor("in_val", [P, si.total_blocks], f32, kind="ExternalInput")

    with tile.TileContext(nc) as tc:
        with (
            tc.tile_pool(name="const", bufs=1) as cp,
            tc.tile_pool(name="persist", bufs=1) as pp,
            tc.tile_pool(name="idx", bufs=8) as idxp,
            tc.tile_pool(name="msgsbf", bufs=12) as mbp,
            tc.tile_pool(name="oh", bufs=6) as ohp,
            tc.tile_pool(name="rl", bufs=6) as rlp,
            tc.tile_pool(name="u0t", bufs=6) as u0p,
            tc.tile_pool(name="drain", bufs=3) as drp,
            tc.tile_pool(name="hrow", bufs=4) as hp,
            tc.tile_pool(name="psacc", bufs=5, space="PSUM") as pap,
            tc.tile_pool(name="pstr", bufs=2, space="PSUM") as ptp,
            tc.tile_pool(name="dram", bufs=1, space="DRAM") as dram,
        ):
            # ---- constants / persistent ----
            iota_i = cp.tile([P, P], mybir.dt.int32, tag="iotai")
            nc.gpsimd.iota(iota_i[:], pattern=[[1, P]], base=0, channel_multiplier=0)
            iota_bf = cp.tile([P, P], bf, tag="iotabf")
            nc.vector.tensor_copy(iota_bf[:], iota_i[:])
            ident = cp.tile([d, d], f32, tag="ident")
            make_identity(nc, ident[:])
            ist_t = pp.tile([P, cfg.t1p], f32, tag="ist")
            nc.sync.dma_start(ist_t[:], t_ist.ap())
            isb_t = pp.tile([P, ubt_p], f32, tag="isb")
            nc.sync.dma_start(isb_t[:], t_isb.ap())
            iib_t = pp.tile([P, ubt_p], f32, tag="iib")
            nc.sync.dma_start(iib_t[:], t_iib.ap())
            t3T = pp.tile([d, ubp], f32, tag="t3T")
            h1b_t = pp.tile([P, ubt_p, d], f32, tag="h1b")
            nc.vector.memzero(h1b_t[:])

            # internal DRAM
            h1ag = [
                dram.tile([cfg.cr, EPAD], bf, tag=f"h1ag{k}", name=f"h1ag{k}")
                for k in range(cfg.agc)
            ]
            h1fb = [
                dram.tile([nc_ * cfg.cr, EPAD], bf, tag=f"h1fb{k}",
                          name=f"h1fb{k}")
                for k in range(cfg.agc)
            ]
            h1_shard = dram.tile([cfg.shard_rows, d], f32, tag="h1shard")
            fu_tab = dram.tile([ubp, EPAD], bf, tag="futab")

            def spmm_region(sched, g, c, rl_t, rl_base, t_idx, table_ap,
                            vw_t, psums, first, last):
                """Emit gather/onehot/matmuls for one (group, chunk) region."""
                nidx = int(sched.region_nidx[g, c])
                if nidx == 0:
                    return
                rb = nidx // P
                w0 = int(sched.idx_off[g, c])
                it = idxp.tile([P, nidx // 16], mybir.dt.int16, tag="idx")
                nc.sync.dma_start(it[:], t_idx.ap()[:, w0 : w0 + nidx // 16])
                mb_t = mbp.tile([P, rb, EPAD], bf, tag="msgsbf")
                for s0 in range(0, nidx, GSUB):
                    n = min(GSUB, nidx - s0)
                    nc.gpsimd.dma_gather(
                        mb_t[:, s0 // P : (s0 + n) // P, :],
                        table_ap,
                        it[:, s0 // 16 : (s0 + n) // 16],
                        n, n, EPAD, single_packet=False,
                        queue_num=next_q(),
                    )
                rboff = (int(sched.blk_base[g, c]) - rl_base)
                if vw_t is not None:
                    nc.vector.tensor_tensor(
                        out=mb_t[:, :, 0:d],
                        in0=mb_t[:, :, 0:d],
                        in1=vw_t[:, rboff : rboff + rb]
                        .unsqueeze(2)
                        .to_broadcast([P, rb, d]),
                        op=mybir.AluOpType.mult,
                    )
                oh_t = ohp.tile([P, rb, P], bf, tag="oh")
                nc.vector.tensor_tensor(
                    out=oh_t[:],
                    in0=rl_t[:, rboff : rboff + rb]
                    .unsqueeze(2)
                    .to_broadcast([P, rb, P]),
                    in1=iota_bf[:].unsqueeze(1).to_broadcast([P, rb, P]),
                    op=mybir.AluOpType.is_equal,
                )
                for j in range(rb):
                    gblk = int(sched.blk_base[g, c]) + j
                    # which tile does this block belong to?
                    soff = j * P
                    tl = 0
                    for tt in range(sched.tpg):
                        t_ = g * sched.tpg + tt
                        if (sched.sub_off[t_, c] <= soff
                                < sched.sub_off[t_, c] + sched.cap[t_, c]):
                            tl = tt
                            break
                    out_ap = psums[tl]
                    nc.tensor.matmul(
                        out_ap,
                        lhsT=mb_t[:, j, 0:d],
                        rhs=oh_t[:, j, :],
                        start=(gblk == first[tl]),
                        stop=(gblk == last[tl]),
                    )

            def startstop(sched, g):
                first = {tl: sched.tile_blocks[(g, tl)][0]
                         for tl in range(sched.tpg) if sched.tile_blocks[(g, tl)]}
                last = {tl: sched.tile_blocks[(g, tl)][-1]
                        for tl in range(sched.tpg) if sched.tile_blocks[(g, tl)]}
                return first, last

            def spmm_group(sched, g, t_idx, t_rl, table_aps, val_t, psums):
                """Emit gathers/onehot/matmuls for one group (all chunks)."""
                gb0 = int(sched.group_blk0[g])
                gblocks = sched.group_blocks[g]
                if gblocks == 0:
                    return
                rl_t = rlp.tile([P, gblocks], bf, tag="rl")
                nc.sync.dma_start(rl_t[:], t_rl.ap()[:, gb0 : gb0 + gblocks])
                vw_t = None
                if val_t is not None:
                    vw_t = rlp.tile([P, gblocks], f32, tag="vw")
                    nc.sync.dma_start(vw_t[:], val_t.ap()[:, gb0 : gb0 + gblocks])
                first, last = startstop(sched, g)
                for c in range(sched.nch):
                    spmm_region(sched, g, c, rl_t, gb0, t_idx, table_aps[c],
                                vw_t, psums, first, last)

            def psum_packs(tpg):
                packs = []
                for i in range(0, tpg, 4):
                    w = min(4, tpg - i) * P
                    packs.append(
                        pap.tile([d, w], f32, tag="acc", name="accpk")
                    )
                return packs

            def tile_psum(packs, tl):
                return packs[tl // 4][:, (tl % 4) * P : (tl % 4 + 1) * P]

            # ================= L1 =================
            user_chunks = [
                t_userbf.ap()[c * cfg.ch_u : min((c + 1) * cfg.ch_u, cfg.n_user), :]
                for c in range(cfg.nch_u)
            ]

            def issue_ag(k):
                nc.gpsimd.collective_compute(
                    "AllGather",
                    mybir.AluOpType.bypass,
                    replica_groups=[list(range(nc_))],
                    ins=[h1ag[k][:].opt()],
                    outs=[h1fb[k][:].opt()],
                )

            ag_issued = 0
            for g in range(s1.ng):
                packs = psum_packs(s1.tpg)
                psums = [tile_psum(packs, tl) for tl in range(s1.tpg)]
                spmm_group(
                    s1, g, t_l1i, t_l1r, user_chunks,
                    t_vals.get("l1"), psums,
                )
                u0_t = u0p.tile([P, s1.tpg, d], f32, tag="u0t")
                r0 = g * s1.tpg * P
                nc.sync.dma_start(
                    u0_t[:],
                    t_u0s.ap()[r0 : r0 + s1.tpg * P, :].rearrange(
                        "(t p) d -> p t d", p=P
                    ),
                )
                drains = []
                for pk in packs:
                    dsb = drp.tile([d, pk.shape[1]], f32, tag="drain")
                    nc.scalar.copy(dsb[:], pk[:])
                    drains.append(dsb)
                for tl in range(s1.tpg):
                    src = drains[tl // 4][:, (tl % 4) * P : (tl % 4 + 1) * P]
                    ptr = ptp.tile([P, d], f32, tag="tr")
                    nc.tensor.transpose(ptr[:], src, ident[:])
                    h1_t = hp.tile([P, d], f32, tag="hrow")
                    gt = g * s1.tpg + tl
                    nc.vector.scalar_tensor_tensor(
                        out=h1_t[:],
                        in0=ptr[:],
                        scalar=ist_t[:, gt : gt + 1],
                        in1=u0_t[:, tl, :],
                        op0=mybir.AluOpType.mult,
                        op1=mybir.AluOpType.add,
                    )
                    k = g // cfg.gpa
                    lrow = ((g % cfg.gpa) * s1.tpg + tl) * P
                    h1_b16 = hp.tile([P, EPAD], bf, tag="hrowb")
                    nc.scalar.copy(h1_b16[:, 0:d], h1_t[:])
                    nc.scalar.dma_start(
                        h1ag[k][lrow : lrow + P, :], h1_b16[:]
                    )
                    nc.scalar.dma_start(
                        h1_shard[gt * P : (gt + 1) * P, :], h1_t[:]
                    )
                # stagger AllGather issue ~6 groups after its data is ready
                # so the Pool queue never stalls on the drain chain.
                if g >= 12 and (g - 12) % cfg.gpa == 0:
                    issue_ag((g - 12) // cfg.gpa)
                    ag_issued += 1

            # ================= INFO =================
            item_chunks = [
                t_itembf.ap()[c * cfg.ch_i : min((c + 1) * cfg.ch_i, cfg.n_item), :]
                for c in range(cfg.nch_i)
            ]
            # run INFO first (independent of AllGather), then L2
            for g in range(si.ng):
                packs = psum_packs(si.tpg)
                psums = [tile_psum(packs, tl) for tl in range(si.tpg)]
                spmm_group(si, g, t_ini, t_inr, item_chunks,
                           t_vals.get("in"), psums)
                for ip, pk in enumerate(packs):
                    o0 = (g * si.tpg + ip * 4) * P
                    nc.scalar.copy(
                        t3T[:, o0 : o0 + pk.shape[1]], pk[:]
                    )

            # remaining AllGathers
            for k in range(ag_issued, cfg.agc):
                issue_ag(k)

            # h1 batch rows gather (from own shard)
            h1bi_t = pp.tile([P, ubp // 16], mybir.dt.int16, tag="h1bidx")
            nc.sync.dma_start(h1bi_t[:], t_h1bi.ap())
            for s0 in range(0, ubp, GSUB):
                n = min(GSUB, ubp - s0)
                nc.gpsimd.dma_gather(
                    h1b_t[:, s0 // P : (s0 + n) // P, :],
                    h1_shard[:],
                    h1bi_t[:, s0 // 16 : (s0 + n) // 16],
                    n, n, d, single_packet=False,
                    queue_num=next_q(),
                )

            # ================= L2 (chunk-major) =================
            h1_chunks = [h1fb[k][:] for k in range(cfg.agc)]
            rl2_t = pp.tile([P, s2.total_blocks], bf, tag="rl2")
            nc.sync.dma_start(rl2_t[:], t_l2r.ap())
            vw2_t = None
            if t_vals.get("l2") is not None:
                vw2_t = pp.tile([P, s2.total_blocks], f32, tag="vw2")
                nc.sync.dma_start(vw2_t[:], t_vals["l2"].ap())
            packs2 = [psum_packs(s2.tpg) for _ in range(s2.ng)]
            psums2 = [
                [tile_psum(packs2[g], tl) for tl in range(s2.tpg)]
                for g in range(s2.ng)
            ]
            ss2 = [startstop(s2, g) for g in range(s2.ng)]
            for c in range(s2.nch):
                for g in range(s2.ng):
                    spmm_region(s2, g, c, rl2_t, 0, t_l2i, h1_chunks[c],
                                vw2_t, psums2[g], ss2[g][0], ss2[g][1])
            for g in range(s2.ng):
                packs = packs2[g]
                drains = []
                for pk in packs:
                    dsb = drp.tile([d, pk.shape[1]], f32, tag="drain")
                    nc.scalar.copy(dsb[:], pk[:])
                    drains.append(dsb)
                for tl in range(s2.tpg):
                    T = g * s2.tpg + tl
                    src = drains[tl // 4][:, (tl % 4) * P : (tl % 4 + 1) * P]
                    ptr = ptp.tile([P, d], f32, tag="tr")
                    nc.tensor.transpose(ptr[:], src, ident[:])
                    x1 = hp.tile([P, d], f32, tag="hrow")
                    nc.vector.tensor_scalar_mul(
                        x1[:], ptr[:], isb_t[:, T : T + 1]
                    )
                    ptr3 = ptp.tile([P, d], f32, tag="tr")
                    nc.tensor.transpose(
                        ptr3[:], t3T[:, T * P : (T + 1) * P], ident[:]
                    )
                    x2 = hp.tile([P, d], f32, tag="hrow")
                    nc.vector.scalar_tensor_tensor(
                        out=x2[:], in0=ptr3[:],
                        scalar=iib_t[:, T : T + 1], in1=x1[:],
                        op0=mybir.AluOpType.mult, op1=mybir.AluOpType.add,
                    )
                    fu = hp.tile([P, d], f32, tag="hrow")
                    nc.vector.scalar_tensor_tensor(
                        out=fu[:], in0=h1b_t[:, T, :], scalar=2.0, in1=x2[:],
                        op0=mybir.AluOpType.mult, op1=mybir.AluOpType.add,
                    )
                    fub = hp.tile([P, EPAD], bf, tag="hrowb")
                    nc.scalar.copy(fub[:, 0:d], fu[:])
                    nc.scalar.dma_start(fu_tab[T * P : (T + 1) * P, :], fub[:])

            # ================= FINAL =================
            sc_t = pp.tile([P, ftot // P], f32, tag="scores")
            for chn in range(cfg.nch_i):
                n = int(fcap[chn])
                s0 = int(fbase[chn])
                fb = n // P
                iu = idxp.tile([P, n // 16], mybir.dt.int16, tag="idx")
                nc.sync.dma_start(
                    iu[:], t_pui.ap()[:, s0 // 16 : (s0 + n) // 16]
                )
                ii = idxp.tile([P, n // 16], mybir.dt.int16, tag="idx")
                nc.sync.dma_start(
                    ii[:], t_pii.ap()[:, s0 // 16 : (s0 + n) // 16]
                )
                u_t = mbp.tile([P, fb, EPAD], bf, tag="msgsbf")
                v_t = mbp.tile([P, fb, EPAD], bf, tag="msgsbf")
                for q0 in range(0, n, GSUB):
                    nq = min(GSUB, n - q0)
                    nc.gpsimd.dma_gather(
                        u_t[:, q0 // P : (q0 + nq) // P, :], fu_tab[:],
                        iu[:, q0 // 16 : (q0 + nq) // 16], nq, nq, EPAD,
                        single_packet=False, queue_num=next_q(),
                    )
                    nc.gpsimd.dma_gather(
                        v_t[:, q0 // P : (q0 + nq) // P, :], item_chunks[chn],
                        ii[:, q0 // 16 : (q0 + nq) // 16], nq, nq, EPAD,
                        single_packet=False, queue_num=next_q(),
                    )
                pr = ohp.tile([P, fb, d], f32, tag="prod")
                nc.vector.tensor_mul(
                    pr[:], u_t[:, :, 0:d], v_t[:, :, 0:d]
                )
                dot = hp.tile([P, fb], f32, tag="dot")
                nc.vector.tensor_reduce(
                    dot[:], pr[:], axis=mybir.AxisListType.X,
                    op=mybir.AluOpType.add,
                )
                nc.scalar.activation(
                    sc_t[:, s0 // P : (s0 + n) // P], dot[:],
                    mybir.ActivationFunctionType.Sigmoid, scale=2.0,
                )
            nc.scalar.dma_start(t_scores.ap(), sc_t[:])

    nc.compile()
    return nc


_CACHE = {}


def _run(cfg, inputs, trace=False):
    import time as _time

    _t = _time.time()
    plan, in_maps, out_meta = _prep(cfg, inputs)
    print(f"[kernel] prep: {_time.time()-_t:.1f}s", flush=True)
    _t = _time.time()
    key = (
        cfg.n_user, plan["s1"].total_slots, plan["s2"].total_slots,
        plan["si"].total_slots, plan["ubt_p"], plan["ftot"], plan["ones"],
    )
    if key not in _CACHE:
        _CACHE[key] = _build_program(plan)
        print(f"[kernel] build+compile: {_time.time()-_t:.1f}s", flush=True)
    nc = _CACHE[key]
    _t = _time.time()
    kw = {}
    if trace:
        # single-core NTFF (SPMD cores are balanced); exec_time_ns comes back
        kw = dict(trace=True, trace_cores=[0])
    res = run_bass_kernel_spmd(
        nc, in_maps, core_ids=list(range(cfg.nc)), **kw
    )
    print(f"[kernel] run: {_time.time()-_t:.1f}s", flush=True)
    out = np.zeros(len(inputs["user_ids"]), np.float32)
    for c in range(cfg.nc):
        js, slots = out_meta[c]
        sc = res.results[c]["scores"]
        out[js] = sc[slots % P, slots // P]
    return out, res


def kernel(**inputs):
    out, _ = _run(REAL, inputs, trace=bool(os.environ.get("KERNEL_TRACE")))
    return out
